# revision 1
# baseline (speedup 1.0000x reference)
"""MinamoTopoModel GAT kernel: host preprocessing + Bass builder.

Self-contained logic module; kernel.py inlines/imports this during dev.
Design (per 8-core SPMD, dst-sharded):
  L1: cnt-histogram trick (host) -> per-group matmuls, no edge gathers.
  L2/L3: per-tile (128-edge) indirect DMA gathers of node records +
         S-matrix (iota-compare) PSUM scatter matmuls, segment softmax
         without max-subtraction, self-loops handled per-group directly.
  Two AllGathers publish per-shard node records between layers.
  Graph pooling -> per-core [50,17] partials; final FC on host.
"""
import numpy as np
import concourse.bacc as bacc
import concourse.bass as bass
import concourse.mybir as mybir
import concourse.tile as tile

F32 = mybir.dt.float32
I32 = mybir.dt.int32
AX = mybir.AxisListType
ALU = mybir.AluOpType
ACT = mybir.ActivationFunctionType
EPS = 1e-5


def host_prep(inputs, N, E, G, NC, TILE=32, EMB=16):
    H1, C1, H2, C2, H3, C3 = 8, 64, 4, 128, 1, 16
    x = np.asarray(inputs['x']).astype(np.int64)
    ei = np.asarray(inputs['edge_index']).astype(np.int64)
    batch = np.asarray(inputs['batch']).astype(np.int64)
    emb = np.asarray(inputs['emb'], np.float32)
    W1 = np.asarray(inputs['W1'], np.float32)
    as1 = np.asarray(inputs['a_src1'], np.float32); ad1 = np.asarray(inputs['a_dst1'], np.float32)
    b1 = np.asarray(inputs['b1'], np.float32)
    g1 = np.asarray(inputs['g1'], np.float32); be1 = np.asarray(inputs['be1'], np.float32)
    W2 = np.asarray(inputs['W2'], np.float32)
    as2 = np.asarray(inputs['a_src2'], np.float32); ad2 = np.asarray(inputs['a_dst2'], np.float32)
    b2 = np.asarray(inputs['b2'], np.float32)
    g2 = np.asarray(inputs['g2'], np.float32); be2 = np.asarray(inputs['be2'], np.float32)
    W3 = np.asarray(inputs['W3'], np.float32)
    as3 = np.asarray(inputs['a_src3'], np.float32); ad3 = np.asarray(inputs['a_dst3'], np.float32)
    b3 = np.asarray(inputs['b3'], np.float32)
    g3 = np.asarray(inputs['g3'], np.float32); be3 = np.asarray(inputs['be3'], np.float32)

    NPC = N // NC                      # nodes per core (exact: 50000/8=6250)
    NG = (NPC + 127) // 128            # groups per core (49)
    NPCP = NG * 128                    # padded nodes per core (6272)

    # ---- L1 tables (cnt trick) ----
    z1 = emb @ W1                                     # [32, 512]
    z1h = z1.reshape(TILE, H1, C1)
    al1t = np.einsum('thc,hc->th', z1h, as1)          # [32,8]
    ar1t = np.einsum('thc,hc->th', z1h, ad1)
    # E_tab[xd, h, t] = exp(lrelu(al1t[t,h] + ar1t[xd,h]))
    ee = al1t.T[None, :, :] + ar1t[:, :, None]        # [xd=32, h=8, t=32]
    ee = np.where(ee > 0, ee, 0.2 * ee)
    E_tab = np.exp(ee).astype(np.float32)             # [32, 8, 32]

    # cnt histogram over ALL edges incl self-loops
    src_all = np.concatenate([ei[0], np.arange(N)])
    dst_all = np.concatenate([ei[1], np.arange(N)])
    xs_all = x[src_all]
    cnt = np.zeros((N, TILE), np.float32)
    np.add.at(cnt, (dst_all, xs_all), 1.0)

    # ---- weight tables ----
    def wprime(W, a_s, a_d, H, C, pad_to):
        Fin = W.shape[0]
        As = np.zeros((H * C, H), np.float32)
        Ad = np.zeros((H * C, H), np.float32)
        for h in range(H):
            As[h * C:(h + 1) * C, h] = a_s[h]
            Ad[h * C:(h + 1) * C, h] = a_d[h]
        Wp = np.concatenate([W, W @ As, W @ Ad], axis=1)  # [Fin, H*C + 2H]
        out = np.zeros((Fin, pad_to), np.float32)
        out[:, :Wp.shape[1]] = Wp
        return out

    REC2 = 576   # 512 z + 8 al + 8 ar + 48 pad (f32)
    REC3 = 32    # 16 z + 1 al + 1 ar + 14 pad
    W2p = wprime(W2, as2, ad2, H2, C2, REC2)          # [512, 576]
    W3p = wprime(W3, as3, ad3, H3, C3, REC3)          # [512, 32]
    W2c = W2p.reshape(4, 128, REC2).copy()
    W3c = W3p.reshape(4, 128, REC3).copy()

    def bc(v, F):
        t = np.zeros((128, F), np.float32); t[:, :] = v[None, :F]; return t

    consts = dict(
        W2c=W2c, W3c=W3c,
        z1t=z1.astype(np.float32),                    # [32, 512]
        b1t=bc(b1, 512), g1t=bc(g1, 512), be1t=bc(be1, 512),
        b2t=bc(b2, 512), g2t=bc(g2, 512), be2t=bc(be2, 512),
        b3t=bc(b3, 16), g3t=bc(g3, 16), be3t=bc(be3, 16),
        iotaF=np.tile(np.arange(128, dtype=np.float32), (128, 1)),
        ident=np.eye(128, dtype=np.float32),
        onesc=np.ones((128, 1), np.float32),
    )

    # ---- per-core edge bucketing (non-self edges only) ----
    es, ed = ei[0], ei[1]
    core_of = ed // NPC
    grp_of = (ed % NPC) // 128
    # count per (core, group)
    counts = np.zeros((NC, NG), np.int64)
    np.add.at(counts, (core_of, grp_of), 1)
    Tg = np.maximum(1, ((counts.max(axis=0) + 127) // 128)).astype(np.int64)  # per-group tiles

    # gather index remap: node n -> row (n//NPC)*NPCP + n%NPC
    gidx_all = (es // NPC) * NPCP + (es % NPC)

    order = np.lexsort((es, grp_of, core_of))
    es_s, ed_s = es[order], ed[order]
    core_s, grp_s = core_of[order], grp_of[order]
    gidx_s = gidx_all[order]
    # boundaries per (core, group)
    starts = np.zeros((NC, NG), np.int64)
    flat = core_s * NG + grp_s
    bounds = np.searchsorted(flat, np.arange(NC * NG))
    starts = bounds.reshape(NC, NG)
    total = len(es_s)

    idx_src = np.zeros((NC, int(Tg.sum()) * 128), np.int32)
    dstloc = np.full((NC, int(Tg.sum()) * 128), 200.0, np.float32)
    toff = np.concatenate([[0], np.cumsum(Tg)]).astype(np.int64)  # tile offsets per group
    for c in range(NC):
        for g in range(NG):
            s = starts[c, g]
            e = starts[c, g + 1] if g + 1 < NG else (starts[c + 1, 0] if c + 1 < NC else total)
            n = e - s
            o = int(toff[g]) * 128
            cap = int(Tg[g]) * 128
            assert n <= cap, (c, g, n, cap)
            idx_src[c, o:o + n] = gidx_s[s:e]
            dstloc[c, o:o + n] = (ed_s[s:e] % NPC) % 128
    # reshape per group tile-major: slot j within group -> (tile j//128? ) We store
    # edge slot j at [tile=j//128 ... wait gather layout: out[p, t] = row idx[t*128+p]
    # => idx array per group laid out [T,128] with tile-major flattening, and the
    # SBUF idx tile loaded as [128, T] must be the transpose.
    NTT = int(Tg.sum())
    idx_src = idx_src.reshape(NC, NTT, 128)
    dstloc = dstloc.reshape(NC, NTT, 128)
    # SBUF-friendly layout [128, NTT]
    idx_srcT = np.ascontiguousarray(idx_src.transpose(0, 2, 1))   # [NC, 128, NTT]
    dstlocT = np.ascontiguousarray(dstloc.transpose(0, 2, 1))     # [NC, 128, NTT]

    # ---- per-core node arrays ----
    percore = []
    for c in range(NC):
        lo, hi = c * NPC, (c + 1) * NPC
        cntc = np.zeros((NPCP, TILE), np.float32)
        cntc[:NPC] = cnt[lo:hi]
        cntc[NPC:, 0] = 1.0  # pad rows: avoid 0/0
        Ec = np.zeros((NPCP, H1 * TILE), np.float32)
        Ec[:NPC] = E_tab[x[lo:hi]].reshape(NPC, H1 * TILE)
        Ec[NPC:] = 1.0
        batchc = np.full((NPCP, 1), 200.0, np.float32)
        batchc[:NPC, 0] = batch[lo:hi]
        percore.append(dict(
            cntc=cntc, Ec=Ec, batchc=batchc,
            idxs=idx_srcT[c], dls=dstlocT[c],
        ))

    meta = dict(N=N, E=E, G=G, NC=NC, NPC=NPC, NG=NG, NPCP=NPCP, Tg=Tg.tolist(),
                toff=toff.tolist(), REC2=REC2, REC3=REC3, H1=H1, C1=C1, H2=H2,
                C2=C2, H3=H3, C3=C3, TILE=TILE)
    host = dict(fcW1=np.asarray(inputs['fcW1'], np.float32),
                fcb1=np.asarray(inputs['fcb1'], np.float32),
                fcW2=np.asarray(inputs['fcW2'], np.float32),
                fcb2=np.asarray(inputs['fcb2'], np.float32),
                batch=batch)
    return consts, percore, meta, host


def layer_norm_elu(nc, pool, y, g_t, be_t, F, epsc=None):
    """In SBUF: y [128,F] -> elu(LN(y)*g+be). In-place heavy; returns new tile."""
    s1 = pool.tile([128, 1], F32, tag="ln_s1")
    nc.vector.tensor_reduce(out=s1[:], in_=y[:], axis=AX.X, op=ALU.add)
    m2 = pool.tile([128, 1], F32, tag="ln_m2")
    nc.vector.tensor_scalar_mul(out=m2[:], in0=s1[:], scalar1=-1.0 / F)
    sq = pool.tile([128, F], F32, tag="ln_sq")
    ss = pool.tile([128, 1], F32, tag="ln_ss")
    nc.scalar.activation(out=sq[:], in_=y[:], func=ACT.Square, bias=m2[:, :1],
                         accum_out=ss[:])
    sd = pool.tile([128, 1], F32, tag="ln_sd")
    nc.scalar.activation(out=sd[:], in_=ss[:], func=ACT.Sqrt, bias=epsc[:, :1], scale=1.0 / F)
    rs = pool.tile([128, 1], F32, tag="ln_rs")
    nc.vector.reciprocal(out=rs[:], in_=sd[:])
    # y <- (y - m) * istd ; then *g ; then +be   (in place)
    nc.vector.tensor_scalar(out=y[:], in0=y[:], scalar1=m2[:, :1], scalar2=rs[:, :1],
                            op0=ALU.add, op1=ALU.mult)
    nc.vector.tensor_tensor(out=y[:], in0=y[:], in1=g_t[:, :F], op=ALU.mult)
    nc.vector.tensor_tensor(out=y[:], in0=y[:], in1=be_t[:, :F], op=ALU.add)
    # ELU = max(x,0) + exp(min(x,0)) - 1 ; sq reused as scratch
    nc.vector.tensor_scalar_min(out=sq[:], in0=y[:], scalar1=0.0)
    nc.scalar.activation(out=sq[:], in_=sq[:], func=ACT.Exp)
    h = pool.tile([128, F], F32, tag="elu_h")
    nc.vector.tensor_scalar(out=h[:], in0=y[:], scalar1=0.0, scalar2=-1.0,
                            op0=ALU.max, op1=ALU.add)
    nc.vector.tensor_tensor(out=h[:], in0=h[:], in1=sq[:], op=ALU.add)
    return h


def transpose_128(nc, sb, pst, src_ap, ident, tag):
    """PE-transpose a [128,128] SBUF slice -> new SBUF tile."""
    pt = pst.tile([128, 128], F32, tag="tp_ps", space="PSUM")
    nc.tensor.transpose(out=pt[:], in_=src_ap, identity=ident[:])
    st = sb.tile([128, 128], F32, tag="tp_sb")
    nc.vector.tensor_copy(out=st[:], in_=pt[:])
    return st


def build(meta):
    NC, NG, NPCP = meta['NC'], meta['NG'], meta['NPCP']
    Tg, toff = meta['Tg'], meta['toff']
    NTT = toff[-1]
    REC2, REC3 = meta['REC2'], meta['REC3']
    G = meta['G']
    TILE, H1 = meta['TILE'], meta['H1']
    NFULL = NC * NPCP

    nc = bacc.Bacc("TRN2", num_devices=NC)
    # inputs
    t_cnt = nc.dram_tensor("cntc", [NPCP, TILE], F32, kind="ExternalInput")
    t_E = nc.dram_tensor("Ec", [NPCP, H1 * TILE], F32, kind="ExternalInput")
    t_bat = nc.dram_tensor("batchc", [NPCP, 1], F32, kind="ExternalInput")
    t_idx = nc.dram_tensor("idxs", [128, NTT], I32, kind="ExternalInput")
    t_dl = nc.dram_tensor("dls", [128, NTT], F32, kind="ExternalInput")
    t_W2c = nc.dram_tensor("W2c", [4, 128, REC2], F32, kind="ExternalInput")
    t_W3c = nc.dram_tensor("W3c", [4, 128, REC3], F32, kind="ExternalInput")
    t_z1t = nc.dram_tensor("z1t", [TILE, 512], F32, kind="ExternalInput")
    cn = {}
    for nm, sh in [("b1t", 512), ("g1t", 512), ("be1t", 512), ("b2t", 512),
                   ("g2t", 512), ("be2t", 512), ("b3t", 16), ("g3t", 16), ("be3t", 16)]:
        cn[nm] = nc.dram_tensor(nm, [128, sh], F32, kind="ExternalInput")
    t_iota = nc.dram_tensor("iotaF", [128, 128], F32, kind="ExternalInput")
    t_id = nc.dram_tensor("ident", [128, 128], F32, kind="ExternalInput")
    t_ones = nc.dram_tensor("onesc", [128, 1], F32, kind="ExternalInput")
    t_out = nc.dram_tensor("part", [G, 17], F32, kind="ExternalOutput")

    with tile.TileContext(nc) as tc:
        with tc.tile_pool(name="const", bufs=1) as cp, \
             tc.tile_pool(name="sb", bufs=2) as sb, \
             tc.tile_pool(name="gbuf", bufs=2) as gb, \
             tc.tile_pool(name="ps", bufs=1, space="PSUM") as ps, \
             tc.tile_pool(name="pst", bufs=2, space="PSUM") as pst, \
             tc.tile_pool(name="pacc", bufs=1, space="PSUM") as pacc, \
             tc.tile_pool(name="dram", bufs=1, space="DRAM") as dp:

            # ---- const loads ----
            C = {}
            for nm, src, shp in [("iotaF", t_iota, [128, 128]), ("ident", t_id, [128, 128]),
                                 ("z1t", t_z1t, [TILE, 512]), ("onesc", t_ones, [128, 1])]:
                C[nm] = cp.tile(shp, F32, tag="c_" + nm, name="c_" + nm)
                nc.sync.dma_start(out=C[nm][:], in_=src[:])
            for nm in cn:
                F = 512 if nm[-2] != '3' else 16
                C[nm] = cp.tile([128, F], F32, tag="c_" + nm, name="c_" + nm)
                nc.sync.dma_start(out=C[nm][:], in_=cn[nm][:])
            W2s = cp.tile([128, 4 * REC2], F32)
            nc.sync.dma_start(out=W2s[:].rearrange("p (a b) -> p a b", a=4), in_=t_W2c[:].rearrange("a p b -> p a b"))
            epsc = cp.tile([128, 1], F32, name="epsc")
            nc.vector.memset(epsc[:], EPS)
            W3s = cp.tile([128, 4 * REC3], F32)
            nc.sync.dma_start(out=W3s[:].rearrange("p (a b) -> p a b", a=4), in_=t_W3c[:].rearrange("a p b -> p a b"))

            rec2_sh = dp.tile([NPCP, REC2], F32)
            rec2_full = dp.tile([NFULL, REC2], F32, addr_space="Shared")
            rec3_sh = dp.tile([NPCP, REC3], F32)
            rec3_full = dp.tile([NFULL, REC3], F32, addr_space="Shared")

            # ================= L1 + phaseA(L2) =================
            for g in range(NG):
                r0 = g * 128
                cg = sb.tile([128, TILE], F32, tag="cg")
                nc.sync.dma_start(out=cg[:], in_=t_cnt[r0:r0 + 128, :])
                Eg = sb.tile([128, H1, TILE], F32, tag="Eg")
                nc.sync.dma_start(out=Eg[:, :, :], in_=t_E[r0:r0 + 128, :].rearrange("p (h t) -> p h t", h=H1))
                M = sb.tile([128, H1, TILE], F32, tag="M")
                nc.vector.tensor_tensor(out=M[:, :, :], in0=Eg[:, :, :],
                                        in1=cg[:, None, :].to_broadcast([128, H1, TILE]),
                                        op=ALU.mult)
                s = sb.tile([128, H1], F32, tag="s")
                nc.vector.tensor_reduce(out=s[:], in_=M[:, :, :], axis=AX.X, op=ALU.add)
                rs = sb.tile([128, H1], F32, tag="rs")
                nc.vector.reciprocal(out=rs[:], in_=s[:])
                nc.vector.tensor_tensor(out=M[:, :, :], in0=M[:, :, :],
                                        in1=rs[:, :, None].to_broadcast([128, H1, TILE]),
                                        op=ALU.mult)
                P = M
                pO = ps.tile([128, 512], F32, tag="pacc_main", space="PSUM")
                for h in range(H1):
                    ptp = pst.tile([128, 128], F32, tag="tp_ps", space="PSUM")
                    nc.tensor.transpose(out=ptp[:TILE, :], in_=P[:, h, :], identity=C["ident"][:])
                    PT = sb.tile([TILE, 128], F32, tag="PT")
                    nc.vector.tensor_copy(out=PT[:], in_=ptp[:TILE, :])
                    nc.tensor.matmul(out=pO[:, h * 64:(h + 1) * 64], lhsT=PT[:],
                                     rhs=C["z1t"][:, h * 64:(h + 1) * 64],
                                     start=True, stop=True)
                y = sb.tile([128, 512], F32, tag="y1")
                nc.vector.tensor_tensor(out=y[:], in0=pO[:], in1=C["b1t"][:], op=ALU.add)
                h1 = layer_norm_elu(nc, sb, y, C["g1t"], C["be1t"], 512, epsc)
                # transpose h1 -> 4 chunks, phase-A W2'
                z2p = ps.tile([128, 512], F32, tag="pz", space="PSUM")
                z2pb = ps.tile([128, 64], F32, tag="z2pb", space="PSUM")
                for k in range(4):
                    hT = transpose_128(nc, sb, pst, h1[:, k * 128:(k + 1) * 128], C["ident"], "h1T")
                    nc.tensor.matmul(out=z2p[:], lhsT=hT[:], rhs=W2s[:, k * REC2:k * REC2 + 512],
                                     start=(k == 0), stop=(k == 3))
                    nc.tensor.matmul(out=z2pb[:], lhsT=hT[:], rhs=W2s[:, k * REC2 + 512:(k + 1) * REC2],
                                     start=(k == 0), stop=(k == 3))
                zs = sb.tile([128, REC2], F32, tag="zs")
                nc.vector.tensor_copy(out=zs[:, :512], in_=z2p[:])
                nc.vector.tensor_copy(out=zs[:, 512:], in_=z2pb[:])
                nc.sync.dma_start(out=rec2_sh[r0:r0 + 128, :], in_=zs[:])

            nc.gpsimd.collective_compute(
                "AllGather", ALU.bypass, replica_groups=[list(range(NC))],
                ins=[rec2_sh.opt()], outs=[rec2_full.opt()])

            # ================= L2 + phaseA(L3) =================
            for g in range(NG):
                r0 = g * 128
                T = Tg[g]
                o0 = toff[g]
                ig = sb.tile([128, T], I32, tag="ig")
                nc.sync.dma_start(out=ig[:], in_=t_idx[:, o0:o0 + T])
                dl = sb.tile([128, T], F32, tag="dl")
                nc.sync.dma_start(out=dl[:], in_=t_dl[:, o0:o0 + T])
                zg = sb.tile([128, REC2], F32, tag="zg")
                nc.sync.dma_start(out=zg[:], in_=rec2_sh[r0:r0 + 128, :])
                Gt = gb.tile([128, T, REC2], F32, tag="G")
                for t in range(T):
                    nc.gpsimd.indirect_dma_start(
                        out=Gt[:, t, :], out_offset=None, in_=rec2_full[:],
                        in_offset=bass.IndirectOffsetOnAxis(ap=ig[:, t:t + 1], axis=0))
                S = gb.tile([128, T, 128], F32, tag="S")
                nc.vector.tensor_tensor(
                    out=S[:, :, :],
                    in0=C["iotaF"][:, None, :].to_broadcast([128, T, 128]),
                    in1=dl[:, :, None].to_broadcast([128, T, 128]),
                    op=ALU.is_equal)
                H2x, C2x = 4, 128
                pAR = ps.tile([128, T * H2x], F32, tag="pAR", space="PSUM")
                for t in range(T):
                    STt = transpose_128(nc, sb, pst, S[:, t, :], C["ident"], "ST")
                    nc.tensor.matmul(out=pAR[:, t * H2x:(t + 1) * H2x], lhsT=STt[:],
                                     rhs=zg[:, 516:520], start=True, stop=True)
                eL = sb.tile([128, T * H2x], F32, tag="eL")
                nc.vector.tensor_tensor(
                    out=eL[:].rearrange("p (t h) -> p t h", h=H2x),
                    in0=Gt[:, :, 512:516], in1=pAR[:].rearrange("p (t h) -> p t h", h=H2x),
                    op=ALU.add)
                eA = sb.tile([128, T * H2x], F32, tag="eA")
                nc.vector.tensor_scalar_mul(out=eA[:], in0=eL[:], scalar1=0.2)
                nc.vector.tensor_tensor(out=eA[:], in0=eL[:], in1=eA[:], op=ALU.max)
                EX = sb.tile([128, T * H2x], F32, tag="EX")
                nc.scalar.activation(out=EX[:], in_=eA[:], func=ACT.Exp)
                # scale z-part of G by EX (per head block of C2x)
                nc.vector.tensor_tensor(
                    out=Gt[:, :, :512].rearrange("p t (h c) -> p t h c", h=H2x),
                    in0=Gt[:, :, :512].rearrange("p t (h c) -> p t h c", h=H2x),
                    in1=EX[:].rearrange("p (t h) -> p t h", h=H2x)[:, :, :, None]
                        .to_broadcast([128, T, H2x, C2x]),
                    op=ALU.mult)
                pMain = ps.tile([128, 512], F32, tag="pacc_main", space="PSUM")
                pS = ps.tile([128, H2x], F32, tag="pacc_s", space="PSUM")
                for t in range(T):
                    nc.tensor.matmul(out=pMain[:], lhsT=S[:, t, :], rhs=Gt[:, t, :512],
                                     start=(t == 0), stop=(t == T - 1))
                    nc.tensor.matmul(out=pS[:], lhsT=S[:, t, :], rhs=EX[:, t * H2x:(t + 1) * H2x],
                                     start=(t == 0), stop=(t == T - 1))
                # self-loop
                eSl = sb.tile([128, H2x], F32, tag="eSl")
                nc.vector.tensor_tensor(out=eSl[:], in0=zg[:, 512:516], in1=zg[:, 516:520], op=ALU.add)
                eSa = sb.tile([128, H2x], F32, tag="eSa")
                nc.vector.tensor_scalar_mul(out=eSa[:], in0=eSl[:], scalar1=0.2)
                nc.vector.tensor_tensor(out=eSa[:], in0=eSl[:], in1=eSa[:], op=ALU.max)
                exS = sb.tile([128, H2x], F32, tag="exS")
                nc.scalar.activation(out=exS[:], in_=eSa[:], func=ACT.Exp)
                selfc = sb.tile([128, 512], F32, tag="selfc")
                nc.vector.tensor_tensor(
                    out=selfc[:].rearrange("p (h c) -> p h c", h=H2x),
                    in0=zg[:, :512].rearrange("p (h c) -> p h c", h=H2x),
                    in1=exS[:, :, None].to_broadcast([128, H2x, C2x]), op=ALU.mult)
                nc.vector.tensor_tensor(out=selfc[:], in0=pMain[:], in1=selfc[:], op=ALU.add)
                sS = sb.tile([128, H2x], F32, tag="sS")
                nc.vector.tensor_tensor(out=sS[:], in0=pS[:], in1=exS[:], op=ALU.add)
                rS = sb.tile([128, H2x], F32, tag="rS")
                nc.vector.reciprocal(out=rS[:], in_=sS[:])
                nc.vector.tensor_tensor(
                    out=selfc[:].rearrange("p (h c) -> p h c", h=H2x),
                    in0=selfc[:].rearrange("p (h c) -> p h c", h=H2x),
                    in1=rS[:, :, None].to_broadcast([128, H2x, C2x]), op=ALU.mult)
                nc.vector.tensor_tensor(out=selfc[:], in0=selfc[:], in1=C["b2t"][:], op=ALU.add)
                h2 = layer_norm_elu(nc, sb, selfc, C["g2t"], C["be2t"], 512, epsc)
                z3p = ps.tile([128, REC3], F32, tag="pz", space="PSUM")
                for k in range(4):
                    hT = transpose_128(nc, sb, pst, h2[:, k * 128:(k + 1) * 128], C["ident"], "h2T")
                    nc.tensor.matmul(out=z3p[:], lhsT=hT[:], rhs=W3s[:, k * REC3:(k + 1) * REC3],
                                     start=(k == 0), stop=(k == 3))
                z3s = sb.tile([128, REC3], F32, tag="z3s")
                nc.vector.tensor_copy(out=z3s[:], in_=z3p[:])
                nc.sync.dma_start(out=rec3_sh[r0:r0 + 128, :], in_=z3s[:])

            nc.gpsimd.collective_compute(
                "AllGather", ALU.bypass, replica_groups=[list(range(NC))],
                ins=[rec3_sh.opt()], outs=[rec3_full.opt()])

            # ================= L3 + pooling =================
            pPool = pacc.tile([128, 17], F32, tag="pPool", space="PSUM")
            for g in range(NG):
                r0 = g * 128
                T = Tg[g]
                o0 = toff[g]
                ig = sb.tile([128, T], I32, tag="ig3")
                nc.sync.dma_start(out=ig[:], in_=t_idx[:, o0:o0 + T])
                dl = sb.tile([128, T], F32, tag="dl3")
                nc.sync.dma_start(out=dl[:], in_=t_dl[:, o0:o0 + T])
                zg = sb.tile([128, REC3], F32, tag="zg3")
                nc.sync.dma_start(out=zg[:], in_=rec3_sh[r0:r0 + 128, :])
                bg = sb.tile([128, 1], F32, tag="bg")
                nc.sync.dma_start(out=bg[:], in_=t_bat[r0:r0 + 128, :])
                Gt = gb.tile([128, T, REC3], F32, tag="G")
                for t in range(T):
                    nc.gpsimd.indirect_dma_start(
                        out=Gt[:, t, :], out_offset=None, in_=rec3_full[:],
                        in_offset=bass.IndirectOffsetOnAxis(ap=ig[:, t:t + 1], axis=0))
                S = gb.tile([128, T, 128], F32, tag="S")
                nc.vector.tensor_tensor(
                    out=S[:, :, :],
                    in0=C["iotaF"][:, None, :].to_broadcast([128, T, 128]),
                    in1=dl[:, :, None].to_broadcast([128, T, 128]),
                    op=ALU.is_equal)
                pAR = ps.tile([128, T], F32, tag="pAR", space="PSUM")
                for t in range(T):
                    STt = transpose_128(nc, sb, pst, S[:, t, :], C["ident"], "ST3")
                    nc.tensor.matmul(out=pAR[:, t:t + 1], lhsT=STt[:],
                                     rhs=zg[:, 17:18], start=True, stop=True)
                eL = sb.tile([128, T], F32, tag="eL3")
                nc.vector.tensor_tensor(out=eL[:], in0=Gt[:, :, 16], in1=pAR[:], op=ALU.add)
                eA = sb.tile([128, T], F32, tag="eA3")
                nc.vector.tensor_scalar_mul(out=eA[:], in0=eL[:], scalar1=0.2)
                nc.vector.tensor_tensor(out=eA[:], in0=eL[:], in1=eA[:], op=ALU.max)
                EX = sb.tile([128, T], F32, tag="EX3")
                nc.scalar.activation(out=EX[:], in_=eA[:], func=ACT.Exp)
                nc.vector.tensor_tensor(
                    out=Gt[:, :, :16], in0=Gt[:, :, :16],
                    in1=EX[:, :, None].to_broadcast([128, T, 16]), op=ALU.mult)
                pM3 = ps.tile([128, 16], F32, tag="pacc_main", space="PSUM")
                pS3 = ps.tile([128, 1], F32, tag="pacc_s", space="PSUM")
                for t in range(T):
                    nc.tensor.matmul(out=pM3[:], lhsT=S[:, t, :], rhs=Gt[:, t, :16],
                                     start=(t == 0), stop=(t == T - 1))
                    nc.tensor.matmul(out=pS3[:], lhsT=S[:, t, :], rhs=EX[:, t:t + 1],
                                     start=(t == 0), stop=(t == T - 1))
                eSl = sb.tile([128, 1], F32, tag="eSl3")
                nc.vector.tensor_tensor(out=eSl[:], in0=zg[:, 16:17], in1=zg[:, 17:18], op=ALU.add)
                eSa = sb.tile([128, 1], F32, tag="eSa3")
                nc.vector.tensor_scalar_mul(out=eSa[:], in0=eSl[:], scalar1=0.2)
                nc.vector.tensor_tensor(out=eSa[:], in0=eSl[:], in1=eSa[:], op=ALU.max)
                exS = sb.tile([128, 1], F32, tag="exS3")
                nc.scalar.activation(out=exS[:], in_=eSa[:], func=ACT.Exp)
                selfc = sb.tile([128, 16], F32, tag="selfc3")
                nc.vector.tensor_scalar(out=selfc[:], in0=zg[:, :16], scalar1=exS[:, :1],
                                        scalar2=None, op0=ALU.mult)
                nc.vector.tensor_tensor(out=selfc[:], in0=pM3[:], in1=selfc[:], op=ALU.add)
                sS = sb.tile([128, 1], F32, tag="sS3")
                nc.vector.tensor_tensor(out=sS[:], in0=pS3[:], in1=exS[:], op=ALU.add)
                rS = sb.tile([128, 1], F32, tag="rS3")
                nc.vector.reciprocal(out=rS[:], in_=sS[:])
                nc.vector.tensor_scalar(out=selfc[:], in0=selfc[:], scalar1=rS[:, :1],
                                        scalar2=None, op0=ALU.mult)
                nc.vector.tensor_tensor(out=selfc[:], in0=selfc[:], in1=C["b3t"][:], op=ALU.add)
                h3 = layer_norm_elu(nc, sb, selfc, C["g3t"], C["be3t"], 16, epsc)
                OB = sb.tile([128, G], F32, tag="OB")
                nc.vector.tensor_tensor(
                    out=OB[:], in0=C["iotaF"][:, :G],
                    in1=bg[:, :1].to_broadcast([128, G]), op=ALU.is_equal)
                h3w = sb.tile([128, 17], F32, tag="h3w")
                nc.vector.tensor_copy(out=h3w[:, :16], in_=h3[:])
                nc.vector.memset(h3w[:, 16:17], 1.0)
                nc.tensor.matmul(out=pPool[:G, :17], lhsT=OB[:], rhs=h3w[:],
                                 start=(g == 0), stop=(g == NG - 1))
            po = sb.tile([128, 17], F32, tag="po")
            nc.vector.tensor_copy(out=po[:G, :], in_=pPool[:G, :])
            nc.sync.dma_start(out=t_out[:, :], in_=po[:G, :])
    nc.finalize()
    return nc


def run(inputs, N, E, G, NC, runner, TILE=32, EMB=16):
    consts, percore, meta, host = host_prep(inputs, N, E, G, NC, TILE, EMB)
    nc = build(meta)
    in_maps = []
    for c in range(NC):
        m = dict(consts)
        m.update(percore[c])
        in_maps.append(m)
    results = runner(nc, in_maps)
    parts = np.stack([r["part"] for r in results])  # [NC, G, 17]
    tot = parts.sum(axis=0)
    pooled = tot[:, :16] / np.maximum(tot[:, 16:17], 1.0)
    h = np.maximum(pooled @ host['fcW1'] + host['fcb1'], 0.0)
    return (h @ host['fcW2'] + host['fcb2']).astype(np.float32)


# ======================= kernel entry =======================
N_FULL, E_FULL, G_FULL, NC_FULL = 50000, 800000, 50, 8
_CACHE = {}


def _hw_runner(nc, in_maps):
    from concourse.bass_utils import run_bass_kernel_spmd
    res = run_bass_kernel_spmd(nc, in_maps, core_ids=list(range(len(in_maps))))
    return res.results


def kernel(**inputs):
    consts, percore, meta, host = host_prep(inputs, N_FULL, E_FULL, G_FULL, NC_FULL)
    key = tuple(meta['Tg'])
    if key not in _CACHE:
        _CACHE[key] = build(meta)
    nc = _CACHE[key]
    in_maps = []
    for c in range(NC_FULL):
        m = dict(consts)
        m.update(percore[c])
        in_maps.append(m)
    results = _hw_runner(nc, in_maps)
    parts = np.stack([r["part"] for r in results])
    tot = parts.sum(axis=0)
    pooled = tot[:, :16] / np.maximum(tot[:, 16:17], 1.0)
    h = np.maximum(pooled @ host['fcW1'] + host['fcb1'], 0.0)
    return (h @ host['fcW2'] + host['fcb2']).astype(np.float32)



# revision 9
# speedup vs baseline: 1.0595x; 1.0595x over previous
"""MinamoTopoModel GAT kernel: host preprocessing + Bass builder (v2, bf16).

Design (8-core SPMD, dst-sharded, bf16 records):
  L1: cnt-histogram trick -> stacked-head matmuls (2 transposes + 2 matmuls),
      LN+ELU, phase-A producing L2 records [z512|al4|ar4] (bf16, 640-elem
      1280B rows) written to 3 local shard tables + a compact attn table.
  Node shards split into 3 tables at group boundaries [0,32,46,49] so each
      table gets ONE AllGather (Shared single-writer) that can start before
      L1 finishes, and every table has <=32768 rows (int16 dma_gather idx).
  L2/L3: per-group batched dma_gather of src records (one per table) +
      batched dst-attn dma_gather from local tables, segment softmax without
      max-subtraction, S-matrix (iota compare) PSUM scatter matmuls,
      self-loops handled per-group directly.
  Graph pooling -> per-core [50,17] partials; final FC on host.
"""
import numpy as np
import ml_dtypes
import concourse.bacc as bacc
import concourse.bass as bass
import concourse.mybir as mybir
import concourse.tile as tile

F32 = mybir.dt.float32
BF16 = mybir.dt.bfloat16
I16 = mybir.dt.int16
AX = mybir.AxisListType
ALU = mybir.AluOpType
ACT = mybir.ActivationFunctionType
EPS = 1e-5
BF = ml_dtypes.bfloat16

N_FULL, E_FULL, G_FULL, NC_FULL = 50000, 800000, 50, 8
NPC = N_FULL // NC_FULL            # 6250
NG = (NPC + 127) // 128            # 49
NPCP = NG * 128                    # 6272
TBOUND = [0, 32, 46, 49]           # table split points (groups)
NTAB = 3
TLO = [b * 128 for b in TBOUND[:-1]]            # local row starts
TSPAN = [(TBOUND[i + 1] - TBOUND[i]) * 128 for i in range(NTAB)]   # 4096,1792,384
REC2 = 640                         # bf16: z512 al4 ar4 pad -> 1280B rows
REC3 = 128                         # bf16: z16 al ar pad -> 256B rows
MAXT = 8                           # tiles per dma_gather (1024-idx HW limit)


def _wrap_idx(flat):
    """softdge idx wrap: flat slot i -> partition i%16, col i//16; x8 copies."""
    n = len(flat)
    assert n % 16 == 0
    w = np.ascontiguousarray(flat.reshape(n // 16, 16).T.astype(np.int16))
    return np.tile(w, (8, 1))


def host_prep(inputs, TILE=32, EMB=16):
    NC = NC_FULL
    H1, C1, H2, C2, H3, C3 = 8, 64, 4, 128, 1, 16
    x = np.asarray(inputs['x']).astype(np.int64)
    ei = np.asarray(inputs['edge_index']).astype(np.int64)
    batch = np.asarray(inputs['batch']).astype(np.int64)
    emb = np.asarray(inputs['emb'], np.float32)
    W1 = np.asarray(inputs['W1'], np.float32)
    as1 = np.asarray(inputs['a_src1'], np.float32); ad1 = np.asarray(inputs['a_dst1'], np.float32)
    b1 = np.asarray(inputs['b1'], np.float32)
    g1 = np.asarray(inputs['g1'], np.float32); be1 = np.asarray(inputs['be1'], np.float32)
    W2 = np.asarray(inputs['W2'], np.float32)
    as2 = np.asarray(inputs['a_src2'], np.float32); ad2 = np.asarray(inputs['a_dst2'], np.float32)
    b2 = np.asarray(inputs['b2'], np.float32)
    g2 = np.asarray(inputs['g2'], np.float32); be2 = np.asarray(inputs['be2'], np.float32)
    W3 = np.asarray(inputs['W3'], np.float32)
    as3 = np.asarray(inputs['a_src3'], np.float32); ad3 = np.asarray(inputs['a_dst3'], np.float32)
    b3 = np.asarray(inputs['b3'], np.float32)
    g3 = np.asarray(inputs['g3'], np.float32); be3 = np.asarray(inputs['be3'], np.float32)

    # ---- L1 tables (cnt trick) ----
    z1 = emb @ W1                                     # [32, 512]
    z1h = z1.reshape(TILE, H1, C1)
    al1t = np.einsum('thc,hc->th', z1h, as1)          # [32,8]
    ar1t = np.einsum('thc,hc->th', z1h, ad1)
    ee = al1t.T[None, :, :] + ar1t[:, :, None]        # [xd=32, h=8, t=32]
    ee = np.where(ee > 0, ee, 0.2 * ee)
    E_tab = np.exp(ee).astype(np.float32)             # [32, 8, 32]

    src_all = np.concatenate([ei[0], np.arange(N_FULL)])
    dst_all = np.concatenate([ei[1], np.arange(N_FULL)])
    xs_all = x[src_all]
    cnt = np.zeros((N_FULL, TILE), np.float32)
    np.add.at(cnt, (dst_all, xs_all), 1.0)

    # ---- Z1 stacked-head tables: Z1A/Z1B [128, 256] ----
    z1b = z1.astype(BF).astype(np.float32)
    Z1A = np.zeros((128, 256), np.float32)
    Z1B = np.zeros((128, 256), np.float32)
    for h in range(4):
        Z1A[h * 32:(h + 1) * 32, h * 64:(h + 1) * 64] = z1b[:, h * 64:(h + 1) * 64]
        Z1B[h * 32:(h + 1) * 32, h * 64:(h + 1) * 64] = z1b[:, (h + 4) * 64:(h + 5) * 64]

    # ---- weight tables W' = [W | W@As | W@Ad] ----
    def wprime(W, a_s, a_d, H, C):
        As = np.zeros((H * C, H), np.float32)
        Ad = np.zeros((H * C, H), np.float32)
        for h in range(H):
            As[h * C:(h + 1) * C, h] = a_s[h]
            Ad[h * C:(h + 1) * C, h] = a_d[h]
        return np.concatenate([W, W @ As, W @ Ad], axis=1)

    W2p = wprime(W2, as2, ad2, H2, C2)                # [512, 520]
    W3p = wprime(W3, as3, ad3, H3, C3)                # [512, 18]
    W2c = np.ascontiguousarray(W2p.reshape(4, 128, 520))
    W3c = np.ascontiguousarray(W3p.reshape(4, 128, 18))

    def bc(v, F):
        t = np.zeros((128, F), np.float32); t[:, :] = v[None, :F]; return t

    consts = dict(
        W2c=W2c.astype(BF), W3c=W3c.astype(BF),
        Z1A=Z1A.astype(BF), Z1B=Z1B.astype(BF),
        b1t=bc(b1, 512), g1t=bc(g1, 512), be1t=bc(be1, 512),
        b2t=bc(b2, 512), g2t=bc(g2, 512), be2t=bc(be2, 512),
        b3t=bc(b3, 16), g3t=bc(g3, 16), be3t=bc(be3, 16),
        iotaF=np.tile(np.arange(128, dtype=np.float32), (128, 1)).astype(BF),
        ident=np.eye(128, dtype=np.float32).astype(BF),
    )

    # ---- per-core edge bucketing (non-self edges only) ----
    es, ed = ei[0], ei[1]
    core_of = ed // NPC
    dr = ed % NPC
    grp_of = dr // 128
    dloc = dr % 128
    sc = es // NPC
    sr = es % NPC
    ti = np.searchsorted(np.array(TLO[1:]), sr, side='right')     # table id 0..2
    tlo = np.array(TLO)[ti]
    gidx = sc * np.array(TSPAN)[ti] + (sr - tlo)                  # row in table

    order = np.lexsort((gidx, ti, grp_of, core_of))
    core_s = core_of[order]; grp_s = grp_of[order]
    ti_s = ti[order]; gidx_s = gidx[order]
    dloc_s = dloc[order]

    key = (core_s * NG + grp_s) * NTAB + ti_s
    bounds = np.searchsorted(key, np.arange(NC * NG * NTAB + 1))
    cntT = (bounds[1:] - bounds[:-1]).reshape(NC, NG, NTAB)
    Tn = np.maximum(1, -(-cntT.max(axis=0) // 128))               # [NG, NTAB]
    Tt = Tn.sum(axis=1)                                           # [NG]
    oN = np.zeros((NTAB, NG + 1), np.int64)
    for t in range(NTAB):
        oN[t, 1:] = np.cumsum(Tn[:, t])
    oT = np.concatenate([[0], np.cumsum(Tt)]).astype(np.int64)
    NTn = [int(oN[t, -1]) for t in range(NTAB)]
    NTT = int(oT[-1])

    percore = []
    for c in range(NC):
        idxN = [np.zeros((128, NTn[t] * 8), np.int16) for t in range(NTAB)]
        idxD = np.zeros((128, NTT * 8), np.int16)
        dlS = np.full((NTT, 128), 200.0, np.float32)
        for g in range(NG):
            r0 = g * 128
            tb = int(oT[g])
            for t in range(NTAB):
                k = (c * NG + g) * NTAB + t
                s, e = bounds[k], bounds[k + 1]
                n = e - s
                cap = int(Tn[g, t]) * 128
                assert n <= cap
                flat = np.zeros(cap, np.int64)
                flat[:n] = gidx_s[s:e]
                dl = np.full(cap, 200.0, np.float32)
                dl[:n] = dloc_s[s:e]
                o = int(oN[t, g])
                idxN[t][:, o * 8:(o + int(Tn[g, t])) * 8] = _wrap_idx(flat)
                dlS[tb:tb + int(Tn[g, t])] = dl.reshape(int(Tn[g, t]), 128)
                dfl = np.full(cap, float(r0), np.float32)
                dfl[:n] = r0 + dloc_s[s:e]
                idxD[:, tb * 8:(tb + int(Tn[g, t])) * 8] = _wrap_idx(dfl.astype(np.int64))
                tb += int(Tn[g, t])
        lo, hi = c * NPC, (c + 1) * NPC
        cntc = np.zeros((NPCP, TILE), np.float32)
        cntc[:NPC] = cnt[lo:hi]
        cntc[NPC:, 0] = 1.0
        Ec = np.zeros((NPCP, H1 * TILE), np.float32)
        Ec[:NPC] = E_tab[x[lo:hi]].reshape(NPC, H1 * TILE)
        Ec[NPC:] = 1.0
        batchc = np.full((NPCP, 1), 200.0, np.float32)
        batchc[:NPC, 0] = batch[lo:hi]
        percore.append(dict(
            cntc=cntc.astype(BF), Ec=Ec.astype(BF),
            batchc=batchc,
            idx0=idxN[0], idx1=idxN[1], idx2=idxN[2], idxD=idxD,
            dlS=np.ascontiguousarray(dlS.T).astype(BF),   # [128, NTT]
        ))

    meta = dict(Tn=Tn.tolist(), Tt=Tt.tolist(),
                oN=oN.tolist(), oT=oT.tolist(),
                NTn=NTn, NTT=NTT)
    host = dict(fcW1=np.asarray(inputs['fcW1'], np.float32),
                fcb1=np.asarray(inputs['fcb1'], np.float32),
                fcW2=np.asarray(inputs['fcW2'], np.float32),
                fcb2=np.asarray(inputs['fcb2'], np.float32))
    return consts, percore, meta, host


def layer_norm_elu(nc, pool, y, g_t, be_t, F, epsc):
    """In SBUF: y f32 [128,F] -> elu(LN(y)*g+be) f32. Returns new tile."""
    s1 = pool.tile([128, 1], F32, tag="ln_s1")
    nc.vector.tensor_reduce(out=s1[:], in_=y[:], axis=AX.X, op=ALU.add)
    m2 = pool.tile([128, 1], F32, tag="ln_m2")
    nc.vector.tensor_scalar_mul(out=m2[:], in0=s1[:], scalar1=-1.0 / F)
    sq = pool.tile([128, F], F32, tag="ln_sq")
    ss = pool.tile([128, 1], F32, tag="ln_ss")
    nc.scalar.activation(out=sq[:], in_=y[:], func=ACT.Square, bias=m2[:, :1],
                         accum_out=ss[:])
    sd = pool.tile([128, 1], F32, tag="ln_sd")
    nc.scalar.activation(out=sd[:], in_=ss[:], func=ACT.Sqrt, bias=epsc[:, :1], scale=1.0 / F)
    rs = pool.tile([128, 1], F32, tag="ln_rs")
    nc.vector.reciprocal(out=rs[:], in_=sd[:])
    nc.vector.tensor_scalar(out=y[:], in0=y[:], scalar1=m2[:, :1], scalar2=rs[:, :1],
                            op0=ALU.add, op1=ALU.mult)
    nc.vector.tensor_tensor(out=y[:], in0=y[:], in1=g_t[:, :F], op=ALU.mult)
    nc.vector.tensor_tensor(out=y[:], in0=y[:], in1=be_t[:, :F], op=ALU.add)
    # ELU = max(x,0) + exp(min(x,0)) - 1
    nc.vector.tensor_scalar_min(out=sq[:], in0=y[:], scalar1=0.0)
    nc.scalar.activation(out=sq[:], in_=sq[:], func=ACT.Exp)
    h = pool.tile([128, F], F32, tag="elu_h")
    nc.vector.tensor_scalar(out=h[:], in0=y[:], scalar1=0.0, scalar2=-1.0,
                            op0=ALU.max, op1=ALU.add)
    nc.vector.tensor_tensor(out=h[:], in0=h[:], in1=sq[:], op=ALU.add)
    return h


def build(meta, phases=3):
    Tn, Tt = meta['Tn'], meta['Tt']
    oN, oT = meta['oN'], meta['oT']
    NTn, NTT = meta['NTn'], meta['NTT']
    NC, G = NC_FULL, G_FULL
    TILE, H1 = 32, 8
    H2x = 4

    def tab_of(g):
        for t in range(NTAB):
            if g < TBOUND[t + 1]:
                return t, (g - TBOUND[t]) * 128
        raise AssertionError

    nc = bacc.Bacc("TRN2", num_devices=NC)
    t_cnt = nc.dram_tensor("cntc", [NPCP, TILE], BF16, kind="ExternalInput")
    t_E = nc.dram_tensor("Ec", [NPCP, H1 * TILE], BF16, kind="ExternalInput")
    t_bat = nc.dram_tensor("batchc", [NPCP, 1], F32, kind="ExternalInput")
    t_iN = [nc.dram_tensor(f"idx{t}", [128, NTn[t] * 8], I16, kind="ExternalInput")
            for t in range(NTAB)]
    t_iD = nc.dram_tensor("idxD", [128, NTT * 8], I16, kind="ExternalInput")
    t_dl = nc.dram_tensor("dlS", [128, NTT], BF16, kind="ExternalInput")
    t_W2c = nc.dram_tensor("W2c", [4, 128, 520], BF16, kind="ExternalInput")
    t_W3c = nc.dram_tensor("W3c", [4, 128, 18], BF16, kind="ExternalInput")
    t_Z1A = nc.dram_tensor("Z1A", [128, 256], BF16, kind="ExternalInput")
    t_Z1B = nc.dram_tensor("Z1B", [128, 256], BF16, kind="ExternalInput")
    cn = {}
    for nm, sh in [("b1t", 512), ("g1t", 512), ("be1t", 512), ("b2t", 512),
                   ("g2t", 512), ("be2t", 512), ("b3t", 16), ("g3t", 16), ("be3t", 16)]:
        cn[nm] = nc.dram_tensor(nm, [128, sh], F32, kind="ExternalInput")
    t_iota = nc.dram_tensor("iotaF", [128, 128], BF16, kind="ExternalInput")
    t_id = nc.dram_tensor("ident", [128, 128], BF16, kind="ExternalInput")
    t_out = nc.dram_tensor("part", [G, 17], F32, kind="ExternalOutput")

    with tile.TileContext(nc) as tc:
        with tc.tile_pool(name="const", bufs=1) as cp, \
             tc.tile_pool(name="sb", bufs=2) as sb, \
             tc.tile_pool(name="gbuf", bufs=2) as gb, \
             tc.tile_pool(name="dbuf", bufs=2) as db, \
             tc.tile_pool(name="ps", bufs=1, space="PSUM") as ps, \
             tc.tile_pool(name="pst", bufs=2, space="PSUM") as pst, \
             tc.tile_pool(name="pacc", bufs=1, space="PSUM") as pacc, \
             tc.tile_pool(name="dram", bufs=1, space="DRAM") as dp:

            # ---- const loads ----
            C = {}
            for nm, src, shp in [("iotaF", t_iota, [128, 128]), ("ident", t_id, [128, 128]),
                                 ("Z1A", t_Z1A, [128, 256]), ("Z1B", t_Z1B, [128, 256])]:
                C[nm] = cp.tile(shp, BF16, tag="c_" + nm, name="c_" + nm)
                nc.sync.dma_start(out=C[nm][:], in_=src[:])
            for nm in cn:
                F = 512 if nm[-2] != '3' else 16
                C[nm] = cp.tile([128, F], F32, tag="c_" + nm, name="c_" + nm)
                nc.sync.dma_start(out=C[nm][:], in_=cn[nm][:])
            W2s = cp.tile([128, 4 * 520], BF16, name="W2s")
            nc.sync.dma_start(out=W2s[:].rearrange("p (a b) -> p a b", a=4),
                              in_=t_W2c[:].rearrange("a p b -> p a b"))
            W3s = cp.tile([128, 4 * 18], BF16, name="W3s")
            nc.sync.dma_start(out=W3s[:].rearrange("p (a b) -> p a b", a=4),
                              in_=t_W3c[:].rearrange("a p b -> p a b"))
            epsc = cp.tile([128, 1], F32, name="epsc")
            nc.vector.memset(epsc[:], EPS)

            rec2_sh = [dp.tile([TSPAN[t], REC2], BF16, name=f"rec2_sh{t}")
                       for t in range(NTAB)]
            rec2_full = [dp.tile([NC * TSPAN[t], REC2], BF16, addr_space="Shared",
                                 name=f"rec2_full{t}") for t in range(NTAB)]
            att2 = dp.tile([NPCP, REC3], BF16, name="att2")
            rec3_sh = [dp.tile([TSPAN[t], REC3], BF16, name=f"rec3_sh{t}")
                       for t in range(NTAB)]
            rec3_full = [dp.tile([NC * TSPAN[t], REC3], BF16, addr_space="Shared",
                                 name=f"rec3_full{t}") for t in range(NTAB)]
            att3 = dp.tile([NPCP, REC3], BF16, name="att3")

            # ================= L1 + phaseA(L2) =================
            for g in range(NG):
                r0 = g * 128
                tb, rk = tab_of(g)
                cg = sb.tile([128, TILE], BF16, tag="cg")
                nc.sync.dma_start(out=cg[:], in_=t_cnt[r0:r0 + 128, :])
                Eg = sb.tile([128, H1, TILE], BF16, tag="Eg")
                nc.sync.dma_start(out=Eg[:, :, :],
                                  in_=t_E[r0:r0 + 128, :].rearrange("p (h t) -> p h t", h=H1))
                M = sb.tile([128, H1, TILE], BF16, tag="M")
                nc.vector.tensor_tensor(out=M[:, :, :], in0=Eg[:, :, :],
                                        in1=cg[:, None, :].to_broadcast([128, H1, TILE]),
                                        op=ALU.mult)
                s = sb.tile([128, H1], F32, tag="s")
                nc.vector.tensor_reduce(out=s[:], in_=M[:, :, :], axis=AX.X, op=ALU.add)
                rs = sb.tile([128, H1], F32, tag="rs")
                nc.vector.reciprocal(out=rs[:], in_=s[:])
                P = sb.tile([128, H1, TILE], BF16, tag="P")
                nc.vector.tensor_tensor(out=P[:, :, :], in0=M[:, :, :],
                                        in1=rs[:, :, None].to_broadcast([128, H1, TILE]),
                                        op=ALU.mult)
                pO = ps.tile([128, 512], F32, tag="pacc_main", space="PSUM")
                for half in range(2):
                    ptp = pst.tile([128, 128], BF16, tag="tp_ps", space="PSUM")
                    nc.tensor.transpose(
                        out=ptp[:],
                        in_=P[:, half * 4:(half + 1) * 4, :].rearrange("p h t -> p (h t)"),
                        identity=C["ident"][:])
                    PT = sb.tile([128, 128], BF16, tag="PT")
                    nc.vector.tensor_copy(out=PT[:], in_=ptp[:])
                    nc.tensor.matmul(out=pO[:, half * 256:(half + 1) * 256], lhsT=PT[:],
                                     rhs=C["Z1A" if half == 0 else "Z1B"][:],
                                     start=True, stop=True)
                y = sb.tile([128, 512], F32, tag="y1")
                nc.vector.tensor_tensor(out=y[:], in0=pO[:], in1=C["b1t"][:], op=ALU.add)
                h1 = layer_norm_elu(nc, sb, y, C["g1t"], C["be1t"], 512, epsc)
                h1b = sb.tile([128, 512], BF16, tag="h1b")
                nc.vector.tensor_copy(out=h1b[:], in_=h1[:])
                z2p = ps.tile([128, 512], F32, tag="pz", space="PSUM")
                z2pb = ps.tile([128, 8], F32, tag="pzb", space="PSUM")
                for q in range(4):
                    ptp = pst.tile([128, 128], BF16, tag="tp_ps", space="PSUM")
                    nc.tensor.transpose(out=ptp[:], in_=h1b[:, q * 128:(q + 1) * 128],
                                        identity=C["ident"][:])
                    hT = sb.tile([128, 128], BF16, tag="hT")
                    nc.vector.tensor_copy(out=hT[:], in_=ptp[:])
                    nc.tensor.matmul(out=z2p[:], lhsT=hT[:], rhs=W2s[:, q * 520:q * 520 + 512],
                                     start=(q == 0), stop=(q == 3))
                    nc.tensor.matmul(out=z2pb[:], lhsT=hT[:], rhs=W2s[:, q * 520 + 512:(q + 1) * 520],
                                     start=(q == 0), stop=(q == 3))
                zs = sb.tile([128, REC2], BF16, tag="zs")
                nc.vector.memset(zs[:, 520:], 0.0)
                nc.vector.tensor_copy(out=zs[:, :512], in_=z2p[:])
                nc.vector.tensor_copy(out=zs[:, 512:520], in_=z2pb[:])
                nc.sync.dma_start(out=rec2_sh[tb][rk:rk + 128, :], in_=zs[:])
                nc.sync.dma_start(out=att2[r0:r0 + 128, :], in_=zs[:, 512:640])
                if g + 1 in TBOUND:
                    nc.gpsimd.collective_compute(
                        "AllGather", ALU.bypass, replica_groups=[list(range(NC))],
                        ins=[rec2_sh[tb].opt()], outs=[rec2_full[tb].opt()])

            # ================= L2 + phaseA(L3) =================
            for g in range(NG if phases >= 2 else 0):
                r0 = g * 128
                tb, rk = tab_of(g)
                T = Tt[g]
                iD = sb.tile([128, T * 8], I16, tag="iD")
                nc.sync.dma_start(out=iD[:], in_=t_iD[:, oT[g] * 8:(oT[g] + T) * 8])
                Gd = db.tile([128, T, REC3], BF16, tag="Gd")
                for c0 in range(0, T, MAXT):
                    cw = min(MAXT, T - c0)
                    nc.gpsimd.dma_gather(
                        out_ap=Gd[:, c0:c0 + cw, :], in_ap=att2[:],
                        idxs_ap=iD[:, c0 * 8:(c0 + cw) * 8],
                        num_idxs=cw * 128, num_idxs_reg=cw * 128, elem_size=REC3)
                iN = []
                for t in range(NTAB):
                    it = sb.tile([128, Tn[g][t] * 8], I16, tag=f"i{t}")
                    nc.sync.dma_start(out=it[:], in_=t_iN[t][:, oN[t][g] * 8:(oN[t][g] + Tn[g][t]) * 8])
                    iN.append(it)
                dl = sb.tile([128, T], BF16, tag="dl")
                nc.sync.dma_start(out=dl[:], in_=t_dl[:, oT[g]:oT[g] + T])
                zg = sb.tile([128, 520], BF16, tag="zg")
                nc.sync.dma_start(out=zg[:], in_=rec2_sh[tb][rk:rk + 128, 0:520])
                Gt = gb.tile([128, T, REC2], BF16, tag="G")
                tacc = 0
                for t in range(NTAB):
                    for c0 in range(0, Tn[g][t], MAXT):
                        cw = min(MAXT, Tn[g][t] - c0)
                        nc.gpsimd.dma_gather(
                            out_ap=Gt[:, tacc + c0:tacc + c0 + cw, :],
                            in_ap=rec2_full[t][:],
                            idxs_ap=iN[t][:, c0 * 8:(c0 + cw) * 8],
                            num_idxs=cw * 128, num_idxs_reg=cw * 128,
                            elem_size=REC2)
                    tacc += Tn[g][t]
                S = gb.tile([128, T, 128], BF16, tag="S")
                nc.vector.tensor_tensor(
                    out=S[:, :, :],
                    in0=C["iotaF"][:, None, :].to_broadcast([128, T, 128]),
                    in1=dl[:, :, None].to_broadcast([128, T, 128]),
                    op=ALU.is_equal)
                eL = sb.tile([128, T, H2x], BF16, tag="eL")
                nc.vector.tensor_tensor(
                    out=eL[:, :, :], in0=Gt[:, :, 512:516],
                    in1=Gd[:, :, 4:8],
                    op=ALU.add)
                eA = sb.tile([128, T * H2x], BF16, tag="eA")
                nc.vector.scalar_tensor_tensor(
                    out=eA[:], in0=eL[:, :, :].rearrange("p t h -> p (t h)"),
                    scalar=0.2, in1=eL[:, :, :].rearrange("p t h -> p (t h)"),
                    op0=ALU.mult, op1=ALU.max)
                EX = sb.tile([128, T * H2x], BF16, tag="EX")
                nc.scalar.activation(out=EX[:], in_=eA[:], func=ACT.Exp)
                nc.vector.tensor_tensor(
                    out=Gt[:, :, :512].rearrange("p t (h c) -> p t h c", h=H2x),
                    in0=Gt[:, :, :512].rearrange("p t (h c) -> p t h c", h=H2x),
                    in1=EX[:].rearrange("p (t h) -> p t h", h=H2x)[:, :, :, None]
                        .to_broadcast([128, T, H2x, 128]),
                    op=ALU.mult)
                pMain = ps.tile([128, 512], F32, tag="pacc_main", space="PSUM")
                pS = ps.tile([128, H2x], F32, tag="pacc_s", space="PSUM")
                for t in range(T):
                    nc.tensor.matmul(out=pMain[:], lhsT=S[:, t, :], rhs=Gt[:, t, :512],
                                     start=(t == 0), stop=(t == T - 1))
                    nc.tensor.matmul(out=pS[:], lhsT=S[:, t, :], rhs=EX[:, t * H2x:(t + 1) * H2x],
                                     start=(t == 0), stop=(t == T - 1))
                # self-loop
                eSl = sb.tile([128, H2x], BF16, tag="eSl")
                nc.vector.tensor_tensor(out=eSl[:], in0=zg[:, 512:516], in1=zg[:, 516:520], op=ALU.add)
                eSa = sb.tile([128, H2x], BF16, tag="eSa")
                nc.vector.scalar_tensor_tensor(out=eSa[:], in0=eSl[:], scalar=0.2,
                                               in1=eSl[:], op0=ALU.mult, op1=ALU.max)
                exS = sb.tile([128, H2x], BF16, tag="exS")
                nc.scalar.activation(out=exS[:], in_=eSa[:], func=ACT.Exp)
                selfz = sb.tile([128, 512], BF16, tag="selfz")
                nc.vector.tensor_tensor(
                    out=selfz[:].rearrange("p (h c) -> p h c", h=H2x),
                    in0=zg[:, :512].rearrange("p (h c) -> p h c", h=H2x),
                    in1=exS[:, :, None].to_broadcast([128, H2x, 128]), op=ALU.mult)
                selfc = sb.tile([128, 512], F32, tag="selfc")
                nc.vector.tensor_tensor(out=selfc[:], in0=pMain[:], in1=selfz[:], op=ALU.add)
                sS = sb.tile([128, H2x], F32, tag="sS")
                nc.vector.tensor_tensor(out=sS[:], in0=pS[:], in1=exS[:], op=ALU.add)
                rS = sb.tile([128, H2x], F32, tag="rS")
                nc.vector.reciprocal(out=rS[:], in_=sS[:])
                nc.vector.tensor_tensor(
                    out=selfc[:].rearrange("p (h c) -> p h c", h=H2x),
                    in0=selfc[:].rearrange("p (h c) -> p h c", h=H2x),
                    in1=rS[:, :, None].to_broadcast([128, H2x, 128]), op=ALU.mult)
                nc.vector.tensor_tensor(out=selfc[:], in0=selfc[:], in1=C["b2t"][:], op=ALU.add)
                h2 = layer_norm_elu(nc, sb, selfc, C["g2t"], C["be2t"], 512, epsc)
                h2b = sb.tile([128, 512], BF16, tag="h2b")
                nc.vector.tensor_copy(out=h2b[:], in_=h2[:])
                z3p = ps.tile([128, 18], F32, tag="pz", space="PSUM")
                for q in range(4):
                    ptp = pst.tile([128, 128], BF16, tag="tp_ps", space="PSUM")
                    nc.tensor.transpose(out=ptp[:], in_=h2b[:, q * 128:(q + 1) * 128],
                                        identity=C["ident"][:])
                    hT = sb.tile([128, 128], BF16, tag="hT")
                    nc.vector.tensor_copy(out=hT[:], in_=ptp[:])
                    nc.tensor.matmul(out=z3p[:], lhsT=hT[:], rhs=W3s[:, q * 18:(q + 1) * 18],
                                     start=(q == 0), stop=(q == 3))
                z3s = sb.tile([128, REC3], BF16, tag="z3s")
                nc.vector.memset(z3s[:, 18:], 0.0)
                nc.vector.tensor_copy(out=z3s[:, :18], in_=z3p[:])
                nc.sync.dma_start(out=rec3_sh[tb][rk:rk + 128, :], in_=z3s[:])
                nc.sync.dma_start(out=att3[r0:r0 + 128, :], in_=z3s[:])
                if g + 1 in TBOUND:
                    nc.gpsimd.collective_compute(
                        "AllGather", ALU.bypass, replica_groups=[list(range(NC))],
                        ins=[rec3_sh[tb].opt()], outs=[rec3_full[tb].opt()])

            # ================= L3 + pooling =================
            pPool = pacc.tile([128, 17], F32, tag="pPool", space="PSUM")
            for g in range(NG if phases >= 3 else 0):
                r0 = g * 128
                tb, rk = tab_of(g)
                T = Tt[g]
                iD = sb.tile([128, T * 8], I16, tag="iD")
                nc.sync.dma_start(out=iD[:], in_=t_iD[:, oT[g] * 8:(oT[g] + T) * 8])
                Gd = db.tile([128, T, REC3], BF16, tag="Gd")
                for c0 in range(0, T, MAXT):
                    cw = min(MAXT, T - c0)
                    nc.gpsimd.dma_gather(
                        out_ap=Gd[:, c0:c0 + cw, :], in_ap=att3[:],
                        idxs_ap=iD[:, c0 * 8:(c0 + cw) * 8],
                        num_idxs=cw * 128, num_idxs_reg=cw * 128, elem_size=REC3)
                iN = []
                for t in range(NTAB):
                    it = sb.tile([128, Tn[g][t] * 8], I16, tag=f"i{t}")
                    nc.sync.dma_start(out=it[:], in_=t_iN[t][:, oN[t][g] * 8:(oN[t][g] + Tn[g][t]) * 8])
                    iN.append(it)
                dl = sb.tile([128, T], BF16, tag="dl")
                nc.sync.dma_start(out=dl[:], in_=t_dl[:, oT[g]:oT[g] + T])
                zg = sb.tile([128, 18], BF16, tag="zg")
                nc.sync.dma_start(out=zg[:], in_=rec3_sh[tb][rk:rk + 128, 0:18])
                bg = sb.tile([128, 1], F32, tag="bg")
                nc.sync.dma_start(out=bg[:], in_=t_bat[r0:r0 + 128, :])
                Gt = gb.tile([128, T, REC3], BF16, tag="G")
                tacc = 0
                for t in range(NTAB):
                    for c0 in range(0, Tn[g][t], MAXT):
                        cw = min(MAXT, Tn[g][t] - c0)
                        nc.gpsimd.dma_gather(
                            out_ap=Gt[:, tacc + c0:tacc + c0 + cw, :],
                            in_ap=rec3_full[t][:],
                            idxs_ap=iN[t][:, c0 * 8:(c0 + cw) * 8],
                            num_idxs=cw * 128, num_idxs_reg=cw * 128,
                            elem_size=REC3)
                    tacc += Tn[g][t]
                S = gb.tile([128, T, 128], BF16, tag="S")
                nc.vector.tensor_tensor(
                    out=S[:, :, :],
                    in0=C["iotaF"][:, None, :].to_broadcast([128, T, 128]),
                    in1=dl[:, :, None].to_broadcast([128, T, 128]),
                    op=ALU.is_equal)
                eL = sb.tile([128, T], BF16, tag="eL")
                nc.vector.tensor_tensor(out=eL[:], in0=Gt[:, :, 16],
                                        in1=Gd[:, :, 17],
                                        op=ALU.add)
                eA = sb.tile([128, T], BF16, tag="eA")
                nc.vector.scalar_tensor_tensor(out=eA[:], in0=eL[:], scalar=0.2,
                                               in1=eL[:], op0=ALU.mult, op1=ALU.max)
                EX = sb.tile([128, T], BF16, tag="EX")
                nc.scalar.activation(out=EX[:], in_=eA[:], func=ACT.Exp)
                nc.vector.tensor_tensor(
                    out=Gt[:, :, :16], in0=Gt[:, :, :16],
                    in1=EX[:, :, None].to_broadcast([128, T, 16]), op=ALU.mult)
                nc.vector.tensor_copy(out=Gt[:, :, 16], in_=EX[:])
                pM = ps.tile([128, 17], F32, tag="pacc_main", space="PSUM")
                for t in range(T):
                    nc.tensor.matmul(out=pM[:], lhsT=S[:, t, :], rhs=Gt[:, t, :17],
                                     start=(t == 0), stop=(t == T - 1))
                eSl = sb.tile([128, 1], BF16, tag="eSl")
                nc.vector.tensor_tensor(out=eSl[:], in0=zg[:, 16:17], in1=zg[:, 17:18], op=ALU.add)
                eSa = sb.tile([128, 1], BF16, tag="eSa")
                nc.vector.scalar_tensor_tensor(out=eSa[:], in0=eSl[:], scalar=0.2,
                                               in1=eSl[:], op0=ALU.mult, op1=ALU.max)
                exS = sb.tile([128, 1], BF16, tag="exS")
                nc.scalar.activation(out=exS[:], in_=eSa[:], func=ACT.Exp)
                selfz = sb.tile([128, 16], BF16, tag="selfz")
                nc.vector.tensor_tensor(out=selfz[:], in0=zg[:, :16],
                                        in1=exS[:, :1].to_broadcast([128, 16]), op=ALU.mult)
                selfc = sb.tile([128, 16], F32, tag="selfc")
                nc.vector.tensor_tensor(out=selfc[:], in0=pM[:, :16], in1=selfz[:], op=ALU.add)
                sS = sb.tile([128, 1], F32, tag="sS")
                nc.vector.tensor_tensor(out=sS[:], in0=pM[:, 16:17], in1=exS[:], op=ALU.add)
                rS = sb.tile([128, 1], F32, tag="rS")
                nc.vector.reciprocal(out=rS[:], in_=sS[:])
                nc.vector.tensor_scalar(out=selfc[:], in0=selfc[:], scalar1=rS[:, :1],
                                        scalar2=None, op0=ALU.mult)
                nc.vector.tensor_tensor(out=selfc[:], in0=selfc[:], in1=C["b3t"][:], op=ALU.add)
                h3 = layer_norm_elu(nc, sb, selfc, C["g3t"], C["be3t"], 16, epsc)
                OB = sb.tile([128, G], BF16, tag="OB")
                nc.vector.tensor_tensor(
                    out=OB[:], in0=C["iotaF"][:, :G],
                    in1=bg[:, :1].to_broadcast([128, G]), op=ALU.is_equal)
                h3w = sb.tile([128, 17], BF16, tag="h3w")
                nc.vector.tensor_copy(out=h3w[:, :16], in_=h3[:])
                nc.vector.memset(h3w[:, 16:17], 1.0)
                nc.tensor.matmul(out=pPool[:G, :17], lhsT=OB[:], rhs=h3w[:],
                                 start=(g == 0), stop=(g == NG - 1))
            po = sb.tile([128, 17], F32, tag="po")
            if phases >= 3:
                nc.vector.tensor_copy(out=po[:G, :], in_=pPool[:G, :])
            else:
                nc.vector.memset(po[:, :], 0.0)
            nc.sync.dma_start(out=t_out[:, :], in_=po[:G, :])
    nc.finalize()
    return nc


_CACHE = {}


def kernel(**inputs):
    consts, percore, meta, host = host_prep(inputs)
    key = tuple(tuple(r) for r in meta['Tn'])
    if key not in _CACHE:
        _CACHE[key] = build(meta)
    nc = _CACHE[key]
    in_maps = []
    for c in range(NC_FULL):
        m = dict(consts)
        m.update(percore[c])
        in_maps.append(m)
    from concourse.bass_utils import run_bass_kernel_spmd
    res = run_bass_kernel_spmd(nc, in_maps, core_ids=list(range(NC_FULL)))
    parts = np.stack([r["part"] for r in res.results])
    tot = parts.sum(axis=0)
    pooled = tot[:, :16] / np.maximum(tot[:, 16:17], 1.0)
    h = np.maximum(pooled @ host['fcW1'] + host['fcb1'], 0.0)
    return (h @ host['fcW2'] + host['fcb2']).astype(np.float32)


# revision 14
# speedup vs baseline: 1.3765x; 1.2992x over previous
"""MinamoTopoModel GAT kernel: host preprocessing + Bass builder (v2, bf16).

Design (8-core SPMD, dst-sharded, bf16 records):
  L1: cnt-histogram trick -> stacked-head matmuls (2 transposes + 2 matmuls),
      LN+ELU, phase-A producing L2 records [z512|al4|ar4] (bf16, 640-elem
      1280B rows) written to 3 local shard tables + a compact attn table.
  Node shards split into 3 tables at group boundaries [0,32,46,49] so each
      table gets ONE AllGather (Shared single-writer) that can start before
      L1 finishes, and every table has <=32768 rows (int16 dma_gather idx).
  L2/L3: per-group batched dma_gather of src records (one per table) +
      batched dst-attn dma_gather from local tables, segment softmax without
      max-subtraction, S-matrix (iota compare) PSUM scatter matmuls,
      self-loops handled per-group directly.
  Graph pooling -> per-core [50,17] partials; final FC on host.
"""
import numpy as np
import ml_dtypes
import concourse.bacc as bacc
import concourse.bass as bass
import concourse.mybir as mybir
import concourse.tile as tile

F32 = mybir.dt.float32
BF16 = mybir.dt.bfloat16
I16 = mybir.dt.int16
AX = mybir.AxisListType
ALU = mybir.AluOpType
ACT = mybir.ActivationFunctionType
EPS = 1e-5
BF = ml_dtypes.bfloat16

N_FULL, E_FULL, G_FULL, NC_FULL = 50000, 800000, 50, 8
NPC = N_FULL // NC_FULL            # 6250
NG = (NPC + 127) // 128            # 49
NPCP = NG * 128                    # 6272
TBOUND = [0, 32, 46, 49]           # table split points (groups)
NTAB = 3
TLO = [b * 128 for b in TBOUND[:-1]]            # local row starts
TSPAN = [(TBOUND[i + 1] - TBOUND[i]) * 128 for i in range(NTAB)]   # 4096,1792,384
REC2 = 640                         # bf16: z512 al4 ar4 pad -> 1280B rows
REC3 = 128                         # bf16: z16 al ar pad -> 256B rows
MAXT = 8                           # tiles per dma_gather (1024-idx HW limit)


def _wrap_idx(flat):
    """softdge idx wrap: flat slot i -> partition i%16, col i//16; x8 copies."""
    n = len(flat)
    assert n % 16 == 0
    w = np.ascontiguousarray(flat.reshape(n // 16, 16).T.astype(np.int16))
    return np.tile(w, (8, 1))


def host_prep(inputs, TILE=32, EMB=16):
    NC = NC_FULL
    H1, C1, H2, C2, H3, C3 = 8, 64, 4, 128, 1, 16
    x = np.asarray(inputs['x']).astype(np.int64)
    ei = np.asarray(inputs['edge_index']).astype(np.int64)
    batch = np.asarray(inputs['batch']).astype(np.int64)
    emb = np.asarray(inputs['emb'], np.float32)
    W1 = np.asarray(inputs['W1'], np.float32)
    as1 = np.asarray(inputs['a_src1'], np.float32); ad1 = np.asarray(inputs['a_dst1'], np.float32)
    b1 = np.asarray(inputs['b1'], np.float32)
    g1 = np.asarray(inputs['g1'], np.float32); be1 = np.asarray(inputs['be1'], np.float32)
    W2 = np.asarray(inputs['W2'], np.float32)
    as2 = np.asarray(inputs['a_src2'], np.float32); ad2 = np.asarray(inputs['a_dst2'], np.float32)
    b2 = np.asarray(inputs['b2'], np.float32)
    g2 = np.asarray(inputs['g2'], np.float32); be2 = np.asarray(inputs['be2'], np.float32)
    W3 = np.asarray(inputs['W3'], np.float32)
    as3 = np.asarray(inputs['a_src3'], np.float32); ad3 = np.asarray(inputs['a_dst3'], np.float32)
    b3 = np.asarray(inputs['b3'], np.float32)
    g3 = np.asarray(inputs['g3'], np.float32); be3 = np.asarray(inputs['be3'], np.float32)

    # ---- L1 tables (cnt trick) ----
    z1 = emb @ W1                                     # [32, 512]
    z1h = z1.reshape(TILE, H1, C1)
    al1t = np.einsum('thc,hc->th', z1h, as1)          # [32,8]
    ar1t = np.einsum('thc,hc->th', z1h, ad1)
    ee = al1t.T[None, :, :] + ar1t[:, :, None]        # [xd=32, h=8, t=32]
    ee = np.where(ee > 0, ee, 0.2 * ee)
    E_tab = np.exp(ee).astype(np.float32)             # [32, 8, 32]

    src_all = np.concatenate([ei[0], np.arange(N_FULL)])
    dst_all = np.concatenate([ei[1], np.arange(N_FULL)])
    xs_all = x[src_all]
    cnt = np.zeros((N_FULL, TILE), np.float32)
    np.add.at(cnt, (dst_all, xs_all), 1.0)

    # ---- Z1 stacked-head tables: Z1A/Z1B [128, 256] ----
    z1b = z1.astype(BF).astype(np.float32)
    Z1A = np.zeros((128, 256), np.float32)
    Z1B = np.zeros((128, 256), np.float32)
    for h in range(4):
        Z1A[h * 32:(h + 1) * 32, h * 64:(h + 1) * 64] = z1b[:, h * 64:(h + 1) * 64]
        Z1B[h * 32:(h + 1) * 32, h * 64:(h + 1) * 64] = z1b[:, (h + 4) * 64:(h + 5) * 64]

    # ---- weight tables W' = [W | W@As | W@Ad] ----
    def wprime(W, a_s, a_d, H, C):
        As = np.zeros((H * C, H), np.float32)
        Ad = np.zeros((H * C, H), np.float32)
        for h in range(H):
            As[h * C:(h + 1) * C, h] = a_s[h]
            Ad[h * C:(h + 1) * C, h] = a_d[h]
        return np.concatenate([W, W @ As, W @ Ad], axis=1)

    W2p = wprime(W2, as2, ad2, H2, C2)                # [512, 520]
    W3p = wprime(W3, as3, ad3, H3, C3)                # [512, 18]
    W2c = np.ascontiguousarray(W2p.reshape(4, 128, 520))
    W3c = np.ascontiguousarray(W3p.reshape(4, 128, 18))

    def bc(v, F):
        t = np.zeros((128, F), np.float32); t[:, :] = v[None, :F]; return t

    consts = dict(
        W2c=W2c.astype(BF), W3c=W3c.astype(BF),
        Z1A=Z1A.astype(BF), Z1B=Z1B.astype(BF),
        b1t=bc(b1, 512), g1t=bc(g1, 512), be1t=bc(be1, 512),
        b2t=bc(b2, 512), g2t=bc(g2, 512), be2t=bc(be2, 512),
        b3t=bc(b3, 16), g3t=bc(g3, 16), be3t=bc(be3, 16),
        iotaF=np.tile(np.arange(128, dtype=np.float32), (128, 1)).astype(BF),
        ident=np.eye(128, dtype=np.float32).astype(BF),
    )

    # ---- per-core edge bucketing (non-self edges only) ----
    es, ed = ei[0], ei[1]
    core_of = ed // NPC
    dr = ed % NPC
    grp_of = dr // 128
    dloc = dr % 128
    sc = es // NPC
    sr = es % NPC
    ti = np.searchsorted(np.array(TLO[1:]), sr, side='right')     # table id 0..2
    tlo = np.array(TLO)[ti]
    gidx = sc * np.array(TSPAN)[ti] + (sr - tlo)                  # row in table

    order = np.lexsort((gidx, ti, grp_of, core_of))
    core_s = core_of[order]; grp_s = grp_of[order]
    ti_s = ti[order]; gidx_s = gidx[order]
    dloc_s = dloc[order]

    key = (core_s * NG + grp_s) * NTAB + ti_s
    bounds = np.searchsorted(key, np.arange(NC * NG * NTAB + 1))
    cntT = (bounds[1:] - bounds[:-1]).reshape(NC, NG, NTAB)
    Tn = np.maximum(1, -(-cntT.max(axis=0) // 128))               # [NG, NTAB]
    Tt = Tn.sum(axis=1)                                           # [NG]
    oN = np.zeros((NTAB, NG + 1), np.int64)
    for t in range(NTAB):
        oN[t, 1:] = np.cumsum(Tn[:, t])
    oT = np.concatenate([[0], np.cumsum(Tt)]).astype(np.int64)
    NTn = [int(oN[t, -1]) for t in range(NTAB)]
    NTT = int(oT[-1])

    percore = []
    for c in range(NC):
        idxN = [np.zeros((128, NTn[t] * 8), np.int16) for t in range(NTAB)]
        idxD = np.zeros((128, NTT * 8), np.int16)
        dlS = np.full((NTT, 128), 200.0, np.float32)
        for g in range(NG):
            r0 = g * 128
            tb = int(oT[g])
            for t in range(NTAB):
                k = (c * NG + g) * NTAB + t
                s, e = bounds[k], bounds[k + 1]
                n = e - s
                cap = int(Tn[g, t]) * 128
                assert n <= cap
                flat = np.zeros(cap, np.int64)
                flat[:n] = gidx_s[s:e]
                dl = np.full(cap, 200.0, np.float32)
                dl[:n] = dloc_s[s:e]
                o = int(oN[t, g])
                idxN[t][:, o * 8:(o + int(Tn[g, t])) * 8] = _wrap_idx(flat)
                dlS[tb:tb + int(Tn[g, t])] = dl.reshape(int(Tn[g, t]), 128)
                dfl = np.full(cap, float(r0), np.float32)
                dfl[:n] = r0 + dloc_s[s:e]
                idxD[:, tb * 8:(tb + int(Tn[g, t])) * 8] = _wrap_idx(dfl.astype(np.int64))
                tb += int(Tn[g, t])
        lo, hi = c * NPC, (c + 1) * NPC
        cntc = np.zeros((NPCP, TILE), np.float32)
        cntc[:NPC] = cnt[lo:hi]
        cntc[NPC:, 0] = 1.0
        Ec = np.zeros((NPCP, H1 * TILE), np.float32)
        Ec[:NPC] = E_tab[x[lo:hi]].reshape(NPC, H1 * TILE)
        Ec[NPC:] = 1.0
        batchc = np.full((NPCP, 1), 200.0, np.float32)
        batchc[:NPC, 0] = batch[lo:hi]
        percore.append(dict(
            cntc=cntc.astype(BF), Ec=Ec.astype(BF),
            batchc=batchc,
            idx0=idxN[0], idx1=idxN[1], idx2=idxN[2], idxD=idxD,
            dlS=np.ascontiguousarray(dlS.T).astype(BF),   # [128, NTT]
        ))

    meta = dict(Tn=Tn.tolist(), Tt=Tt.tolist(),
                oN=oN.tolist(), oT=oT.tolist(),
                NTn=NTn, NTT=NTT)
    host = dict(fcW1=np.asarray(inputs['fcW1'], np.float32),
                fcb1=np.asarray(inputs['fcb1'], np.float32),
                fcW2=np.asarray(inputs['fcW2'], np.float32),
                fcb2=np.asarray(inputs['fcb2'], np.float32))
    return consts, percore, meta, host


def layer_norm_elu(nc, pool, y, g_t, be_t, F, epsc):
    """In SBUF: y f32 [128,F] -> elu(LN(y)*g+be) f32. Returns new tile."""
    s1 = pool.tile([128, 1], F32, tag="ln_s1")
    nc.vector.tensor_reduce(out=s1[:], in_=y[:], axis=AX.X, op=ALU.add)
    m2 = pool.tile([128, 1], F32, tag="ln_m2")
    nc.vector.tensor_scalar_mul(out=m2[:], in0=s1[:], scalar1=-1.0 / F)
    sq = pool.tile([128, F], F32, tag="ln_sq")
    ss = pool.tile([128, 1], F32, tag="ln_ss")
    nc.scalar.activation(out=sq[:], in_=y[:], func=ACT.Square, bias=m2[:, :1],
                         accum_out=ss[:])
    sd = pool.tile([128, 1], F32, tag="ln_sd")
    nc.scalar.activation(out=sd[:], in_=ss[:], func=ACT.Sqrt, bias=epsc[:, :1], scale=1.0 / F)
    rs = pool.tile([128, 1], F32, tag="ln_rs")
    nc.vector.reciprocal(out=rs[:], in_=sd[:])
    nc.vector.tensor_scalar(out=y[:], in0=y[:], scalar1=m2[:, :1], scalar2=rs[:, :1],
                            op0=ALU.add, op1=ALU.mult)
    nc.vector.tensor_tensor(out=y[:], in0=y[:], in1=g_t[:, :F], op=ALU.mult)
    nc.vector.tensor_tensor(out=y[:], in0=y[:], in1=be_t[:, :F], op=ALU.add)
    # ELU = max(x,0) + exp(min(x,0)) - 1
    nc.vector.tensor_scalar_min(out=sq[:], in0=y[:], scalar1=0.0)
    nc.scalar.activation(out=sq[:], in_=sq[:], func=ACT.Exp)
    h = pool.tile([128, F], F32, tag="elu_h")
    nc.vector.tensor_scalar(out=h[:], in0=y[:], scalar1=0.0, scalar2=-1.0,
                            op0=ALU.max, op1=ALU.add)
    nc.vector.tensor_tensor(out=h[:], in0=h[:], in1=sq[:], op=ALU.add)
    return h


def build(meta, phases=3):
    Tn, Tt = meta['Tn'], meta['Tt']
    oN, oT = meta['oN'], meta['oT']
    NTn, NTT = meta['NTn'], meta['NTT']
    NC, G = NC_FULL, G_FULL
    TILE, H1 = 32, 8
    H2x = 4

    def tab_of(g):
        for t in range(NTAB):
            if g < TBOUND[t + 1]:
                return t, (g - TBOUND[t]) * 128
        raise AssertionError

    nc = bacc.Bacc("TRN2", num_devices=NC, num_swdge_queues=4)
    t_cnt = nc.dram_tensor("cntc", [NPCP, TILE], BF16, kind="ExternalInput")
    t_E = nc.dram_tensor("Ec", [NPCP, H1 * TILE], BF16, kind="ExternalInput")
    t_bat = nc.dram_tensor("batchc", [NPCP, 1], F32, kind="ExternalInput")
    t_iN = [nc.dram_tensor(f"idx{t}", [128, NTn[t] * 8], I16, kind="ExternalInput")
            for t in range(NTAB)]
    t_iD = nc.dram_tensor("idxD", [128, NTT * 8], I16, kind="ExternalInput")
    t_dl = nc.dram_tensor("dlS", [128, NTT], BF16, kind="ExternalInput")
    t_W2c = nc.dram_tensor("W2c", [4, 128, 520], BF16, kind="ExternalInput")
    t_W3c = nc.dram_tensor("W3c", [4, 128, 18], BF16, kind="ExternalInput")
    t_Z1A = nc.dram_tensor("Z1A", [128, 256], BF16, kind="ExternalInput")
    t_Z1B = nc.dram_tensor("Z1B", [128, 256], BF16, kind="ExternalInput")
    cn = {}
    for nm, sh in [("b1t", 512), ("g1t", 512), ("be1t", 512), ("b2t", 512),
                   ("g2t", 512), ("be2t", 512), ("b3t", 16), ("g3t", 16), ("be3t", 16)]:
        cn[nm] = nc.dram_tensor(nm, [128, sh], F32, kind="ExternalInput")
    t_iota = nc.dram_tensor("iotaF", [128, 128], BF16, kind="ExternalInput")
    t_id = nc.dram_tensor("ident", [128, 128], BF16, kind="ExternalInput")
    t_out = nc.dram_tensor("part", [G, 17], F32, kind="ExternalOutput")

    with tile.TileContext(nc) as tc:
        with tc.tile_pool(name="const", bufs=1) as cp, \
             tc.tile_pool(name="sb", bufs=2) as sb, \
             tc.tile_pool(name="gbuf", bufs=2) as gb, \
             tc.tile_pool(name="dbuf", bufs=2) as db, \
             tc.tile_pool(name="ps", bufs=1, space="PSUM") as ps, \
             tc.tile_pool(name="pst", bufs=2, space="PSUM") as pst, \
             tc.tile_pool(name="pacc", bufs=1, space="PSUM") as pacc, \
             tc.tile_pool(name="dram", bufs=1, space="DRAM") as dp:

            # ---- const loads ----
            C = {}
            for nm, src, shp in [("iotaF", t_iota, [128, 128]), ("ident", t_id, [128, 128]),
                                 ("Z1A", t_Z1A, [128, 256]), ("Z1B", t_Z1B, [128, 256])]:
                C[nm] = cp.tile(shp, BF16, tag="c_" + nm, name="c_" + nm)
                nc.sync.dma_start(out=C[nm][:], in_=src[:])
            for nm in cn:
                F = 512 if nm[-2] != '3' else 16
                C[nm] = cp.tile([128, F], F32, tag="c_" + nm, name="c_" + nm)
                nc.sync.dma_start(out=C[nm][:], in_=cn[nm][:])
            W2s = cp.tile([128, 4 * 520], BF16, name="W2s")
            nc.sync.dma_start(out=W2s[:].rearrange("p (a b) -> p a b", a=4),
                              in_=t_W2c[:].rearrange("a p b -> p a b"))
            W3s = cp.tile([128, 4 * 18], BF16, name="W3s")
            nc.sync.dma_start(out=W3s[:].rearrange("p (a b) -> p a b", a=4),
                              in_=t_W3c[:].rearrange("a p b -> p a b"))
            epsc = cp.tile([128, 1], F32, name="epsc")
            nc.vector.memset(epsc[:], EPS)

            def gq(out_ap, in_ap, idxs_ap, ni, elem):
                nc.gpsimd.dma_gather(
                    out_ap=out_ap, in_ap=in_ap, idxs_ap=idxs_ap,
                    num_idxs=ni, num_idxs_reg=ni, elem_size=elem)

            def fire():
                pass

            def await_gathers():
                pass

            IN = []
            for t in range(NTAB):
                it = cp.tile([128, NTn[t] * 8], I16, name=f"c_idx{t}")
                nc.sync.dma_start(out=it[:], in_=t_iN[t][:])
                IN.append(it)
            ID = cp.tile([128, NTT * 8], I16, name="c_idxD")
            nc.sync.dma_start(out=ID[:], in_=t_iD[:])
            DL = cp.tile([128, NTT], BF16, name="c_dl")
            nc.sync.dma_start(out=DL[:], in_=t_dl[:])

            rec2_sh = [dp.tile([TSPAN[t], REC2], BF16, name=f"rec2_sh{t}")
                       for t in range(NTAB)]
            rec2_full = [dp.tile([NC * TSPAN[t], REC2], BF16, addr_space="Shared",
                                 name=f"rec2_full{t}") for t in range(NTAB)]
            att2 = dp.tile([NPCP, REC3], BF16, name="att2")
            rec3_sh = [dp.tile([TSPAN[t], REC3], BF16, name=f"rec3_sh{t}")
                       for t in range(NTAB)]
            rec3_full = [dp.tile([NC * TSPAN[t], REC3], BF16, addr_space="Shared",
                                 name=f"rec3_full{t}") for t in range(NTAB)]
            att3 = dp.tile([NPCP, REC3], BF16, name="att3")

            # ================= L1 + phaseA(L2) =================
            for g in range(NG):
                r0 = g * 128
                tb, rk = tab_of(g)
                cg = sb.tile([128, TILE], BF16, tag="cg")
                nc.sync.dma_start(out=cg[:], in_=t_cnt[r0:r0 + 128, :])
                Eg = sb.tile([128, H1, TILE], BF16, tag="Eg")
                nc.sync.dma_start(out=Eg[:, :, :],
                                  in_=t_E[r0:r0 + 128, :].rearrange("p (h t) -> p h t", h=H1))
                M = sb.tile([128, H1, TILE], BF16, tag="M")
                nc.vector.tensor_tensor(out=M[:, :, :], in0=Eg[:, :, :],
                                        in1=cg[:, None, :].to_broadcast([128, H1, TILE]),
                                        op=ALU.mult)
                s = sb.tile([128, H1], F32, tag="s")
                nc.vector.tensor_reduce(out=s[:], in_=M[:, :, :], axis=AX.X, op=ALU.add)
                rs = sb.tile([128, H1], F32, tag="rs")
                nc.vector.reciprocal(out=rs[:], in_=s[:])
                P = sb.tile([128, H1, TILE], BF16, tag="P")
                nc.vector.tensor_tensor(out=P[:, :, :], in0=M[:, :, :],
                                        in1=rs[:, :, None].to_broadcast([128, H1, TILE]),
                                        op=ALU.mult)
                pO = ps.tile([128, 512], F32, tag="pacc_main", space="PSUM")
                for half in range(2):
                    ptp = pst.tile([128, 128], BF16, tag="tp_ps", space="PSUM")
                    nc.tensor.transpose(
                        out=ptp[:],
                        in_=P[:, half * 4:(half + 1) * 4, :].rearrange("p h t -> p (h t)"),
                        identity=C["ident"][:])
                    PT = sb.tile([128, 128], BF16, tag="PT")
                    nc.vector.tensor_copy(out=PT[:], in_=ptp[:])
                    nc.tensor.matmul(out=pO[:, half * 256:(half + 1) * 256], lhsT=PT[:],
                                     rhs=C["Z1A" if half == 0 else "Z1B"][:],
                                     start=True, stop=True)
                y = sb.tile([128, 512], F32, tag="y1")
                nc.vector.tensor_tensor(out=y[:], in0=pO[:], in1=C["b1t"][:], op=ALU.add)
                h1 = layer_norm_elu(nc, sb, y, C["g1t"], C["be1t"], 512, epsc)
                h1b = sb.tile([128, 512], BF16, tag="h1b")
                nc.vector.tensor_copy(out=h1b[:], in_=h1[:])
                z2p = ps.tile([128, 512], F32, tag="pz", space="PSUM")
                z2pb = ps.tile([128, 8], F32, tag="pzb", space="PSUM")
                for q in range(4):
                    ptp = pst.tile([128, 128], BF16, tag="tp_ps", space="PSUM")
                    nc.tensor.transpose(out=ptp[:], in_=h1b[:, q * 128:(q + 1) * 128],
                                        identity=C["ident"][:])
                    hT = sb.tile([128, 128], BF16, tag="hT")
                    nc.vector.tensor_copy(out=hT[:], in_=ptp[:])
                    nc.tensor.matmul(out=z2p[:], lhsT=hT[:], rhs=W2s[:, q * 520:q * 520 + 512],
                                     start=(q == 0), stop=(q == 3))
                    nc.tensor.matmul(out=z2pb[:], lhsT=hT[:], rhs=W2s[:, q * 520 + 512:(q + 1) * 520],
                                     start=(q == 0), stop=(q == 3))
                zs = sb.tile([128, REC2], BF16, tag="zs")
                nc.vector.memset(zs[:, 520:], 0.0)
                nc.vector.tensor_copy(out=zs[:, :512], in_=z2p[:])
                nc.vector.tensor_copy(out=zs[:, 512:520], in_=z2pb[:])
                nc.sync.dma_start(out=rec2_sh[tb][rk:rk + 128, :], in_=zs[:])
                nc.sync.dma_start(out=att2[r0:r0 + 128, :], in_=zs[:, 512:640])
                if g + 1 in TBOUND:
                    nc.gpsimd.collective_compute(
                        "AllGather", ALU.bypass, replica_groups=[list(range(NC))],
                        ins=[rec2_sh[tb].opt()], outs=[rec2_full[tb].opt()])

            # ================= L2 + phaseA(L3) =================
            for g in range(NG if phases >= 2 else 0):
                r0 = g * 128
                tb, rk = tab_of(g)
                T = Tt[g]
                o0 = oT[g]
                dl = DL[:, o0:o0 + T]
                zg = sb.tile([128, 520], BF16, tag="zg")
                nc.sync.dma_start(out=zg[:], in_=rec2_sh[tb][rk:rk + 128, 0:520])
                Gt = gb.tile([128, T, REC2], BF16, tag="G")
                tacc = 0
                for t in range(NTAB):
                    for c0 in range(0, Tn[g][t], MAXT):
                        cw = min(MAXT, Tn[g][t] - c0)
                        gq(Gt[:, tacc + c0:tacc + c0 + cw, :], rec2_full[t][:],
                           IN[t][:, (oN[t][g] + c0) * 8:(oN[t][g] + c0 + cw) * 8],
                           cw * 128, REC2)
                    tacc += Tn[g][t]
                fire()
                S = gb.tile([128, T, 128], BF16, tag="S")
                nc.vector.tensor_tensor(
                    out=S[:, :, :],
                    in0=C["iotaF"][:, None, :].to_broadcast([128, T, 128]),
                    in1=dl[:, :, None].to_broadcast([128, T, 128]),
                    op=ALU.is_equal)
                pAR = ps.tile([128, T * H2x], F32, tag="pAR", space="PSUM")
                for t in range(T):
                    ptp = pst.tile([128, 128], BF16, tag="tp_ps", space="PSUM")
                    nc.tensor.transpose(out=ptp[:], in_=S[:, t, :], identity=C["ident"][:])
                    STt = sb.tile([128, 128], BF16, tag="STt")
                    nc.vector.tensor_copy(out=STt[:], in_=ptp[:])
                    nc.tensor.matmul(out=pAR[:, t * H2x:(t + 1) * H2x], lhsT=STt[:],
                                     rhs=zg[:, 516:520], start=True, stop=True)
                eL = sb.tile([128, T, H2x], BF16, tag="eL")
                nc.vector.tensor_tensor(
                    out=eL[:, :, :], in0=Gt[:, :, 512:516],
                    in1=pAR[:].rearrange("p (t h) -> p t h", h=H2x),
                    op=ALU.add)
                eA = sb.tile([128, T * H2x], BF16, tag="eA")
                nc.vector.scalar_tensor_tensor(
                    out=eA[:], in0=eL[:, :, :].rearrange("p t h -> p (t h)"),
                    scalar=0.2, in1=eL[:, :, :].rearrange("p t h -> p (t h)"),
                    op0=ALU.mult, op1=ALU.max)
                EX = sb.tile([128, T * H2x], BF16, tag="EX")
                nc.scalar.activation(out=EX[:], in_=eA[:], func=ACT.Exp)
                nc.vector.tensor_tensor(
                    out=Gt[:, :, :512].rearrange("p t (h c) -> p t h c", h=H2x),
                    in0=Gt[:, :, :512].rearrange("p t (h c) -> p t h c", h=H2x),
                    in1=EX[:].rearrange("p (t h) -> p t h", h=H2x)[:, :, :, None]
                        .to_broadcast([128, T, H2x, 128]),
                    op=ALU.mult)
                pMain = ps.tile([128, 512], F32, tag="pacc_main", space="PSUM")
                pS = ps.tile([128, H2x], F32, tag="pacc_s", space="PSUM")
                for t in range(T):
                    nc.tensor.matmul(out=pMain[:], lhsT=S[:, t, :], rhs=Gt[:, t, :512],
                                     start=(t == 0), stop=(t == T - 1))
                    nc.tensor.matmul(out=pS[:], lhsT=S[:, t, :], rhs=EX[:, t * H2x:(t + 1) * H2x],
                                     start=(t == 0), stop=(t == T - 1))
                # self-loop
                eSl = sb.tile([128, H2x], BF16, tag="eSl")
                nc.vector.tensor_tensor(out=eSl[:], in0=zg[:, 512:516], in1=zg[:, 516:520], op=ALU.add)
                eSa = sb.tile([128, H2x], BF16, tag="eSa")
                nc.vector.scalar_tensor_tensor(out=eSa[:], in0=eSl[:], scalar=0.2,
                                               in1=eSl[:], op0=ALU.mult, op1=ALU.max)
                exS = sb.tile([128, H2x], BF16, tag="exS")
                nc.scalar.activation(out=exS[:], in_=eSa[:], func=ACT.Exp)
                selfz = sb.tile([128, 512], BF16, tag="selfz")
                nc.vector.tensor_tensor(
                    out=selfz[:].rearrange("p (h c) -> p h c", h=H2x),
                    in0=zg[:, :512].rearrange("p (h c) -> p h c", h=H2x),
                    in1=exS[:, :, None].to_broadcast([128, H2x, 128]), op=ALU.mult)
                selfc = sb.tile([128, 512], F32, tag="selfc")
                nc.vector.tensor_tensor(out=selfc[:], in0=pMain[:], in1=selfz[:], op=ALU.add)
                sS = sb.tile([128, H2x], F32, tag="sS")
                nc.vector.tensor_tensor(out=sS[:], in0=pS[:], in1=exS[:], op=ALU.add)
                rS = sb.tile([128, H2x], F32, tag="rS")
                nc.vector.reciprocal(out=rS[:], in_=sS[:])
                nc.vector.tensor_tensor(
                    out=selfc[:].rearrange("p (h c) -> p h c", h=H2x),
                    in0=selfc[:].rearrange("p (h c) -> p h c", h=H2x),
                    in1=rS[:, :, None].to_broadcast([128, H2x, 128]), op=ALU.mult)
                nc.vector.tensor_tensor(out=selfc[:], in0=selfc[:], in1=C["b2t"][:], op=ALU.add)
                h2 = layer_norm_elu(nc, sb, selfc, C["g2t"], C["be2t"], 512, epsc)
                h2b = sb.tile([128, 512], BF16, tag="h2b")
                nc.vector.tensor_copy(out=h2b[:], in_=h2[:])
                z3p = ps.tile([128, 18], F32, tag="pz", space="PSUM")
                for q in range(4):
                    ptp = pst.tile([128, 128], BF16, tag="tp_ps", space="PSUM")
                    nc.tensor.transpose(out=ptp[:], in_=h2b[:, q * 128:(q + 1) * 128],
                                        identity=C["ident"][:])
                    hT = sb.tile([128, 128], BF16, tag="hT")
                    nc.vector.tensor_copy(out=hT[:], in_=ptp[:])
                    nc.tensor.matmul(out=z3p[:], lhsT=hT[:], rhs=W3s[:, q * 18:(q + 1) * 18],
                                     start=(q == 0), stop=(q == 3))
                z3s = sb.tile([128, REC3], BF16, tag="z3s")
                nc.vector.memset(z3s[:, 18:], 0.0)
                nc.vector.tensor_copy(out=z3s[:, :18], in_=z3p[:])
                nc.sync.dma_start(out=rec3_sh[tb][rk:rk + 128, :], in_=z3s[:])
                nc.sync.dma_start(out=att3[r0:r0 + 128, :], in_=z3s[:])
                if g + 1 in TBOUND:
                    nc.gpsimd.collective_compute(
                        "AllGather", ALU.bypass, replica_groups=[list(range(NC))],
                        ins=[rec3_sh[tb].opt()], outs=[rec3_full[tb].opt()])

            # ================= L3 + pooling =================
            pPool = pacc.tile([128, 17], F32, tag="pPool", space="PSUM")
            for g in range(NG if phases >= 3 else 0):
                r0 = g * 128
                tb, rk = tab_of(g)
                T = Tt[g]
                o0 = oT[g]
                dl = DL[:, o0:o0 + T]
                zg = sb.tile([128, 18], BF16, tag="zg")
                nc.sync.dma_start(out=zg[:], in_=rec3_sh[tb][rk:rk + 128, 0:18])
                bg = sb.tile([128, 1], F32, tag="bg")
                nc.sync.dma_start(out=bg[:], in_=t_bat[r0:r0 + 128, :])
                Gt = gb.tile([128, T, REC3], BF16, tag="G")
                tacc = 0
                for t in range(NTAB):
                    for c0 in range(0, Tn[g][t], MAXT):
                        cw = min(MAXT, Tn[g][t] - c0)
                        gq(Gt[:, tacc + c0:tacc + c0 + cw, :], rec3_full[t][:],
                           IN[t][:, (oN[t][g] + c0) * 8:(oN[t][g] + c0 + cw) * 8],
                           cw * 128, REC3)
                    tacc += Tn[g][t]
                fire()
                S = gb.tile([128, T, 128], BF16, tag="S")
                nc.vector.tensor_tensor(
                    out=S[:, :, :],
                    in0=C["iotaF"][:, None, :].to_broadcast([128, T, 128]),
                    in1=dl[:, :, None].to_broadcast([128, T, 128]),
                    op=ALU.is_equal)
                pAR = ps.tile([128, T], F32, tag="pAR", space="PSUM")
                for t in range(T):
                    ptp = pst.tile([128, 128], BF16, tag="tp_ps", space="PSUM")
                    nc.tensor.transpose(out=ptp[:], in_=S[:, t, :], identity=C["ident"][:])
                    STt = sb.tile([128, 128], BF16, tag="STt")
                    nc.vector.tensor_copy(out=STt[:], in_=ptp[:])
                    nc.tensor.matmul(out=pAR[:, t:t + 1], lhsT=STt[:],
                                     rhs=zg[:, 17:18], start=True, stop=True)
                eL = sb.tile([128, T], BF16, tag="eL")
                nc.vector.tensor_tensor(out=eL[:], in0=Gt[:, :, 16],
                                        in1=pAR[:],
                                        op=ALU.add)
                eA = sb.tile([128, T], BF16, tag="eA")
                nc.vector.scalar_tensor_tensor(out=eA[:], in0=eL[:], scalar=0.2,
                                               in1=eL[:], op0=ALU.mult, op1=ALU.max)
                EX = sb.tile([128, T], BF16, tag="EX")
                nc.scalar.activation(out=EX[:], in_=eA[:], func=ACT.Exp)
                nc.vector.tensor_tensor(
                    out=Gt[:, :, :16], in0=Gt[:, :, :16],
                    in1=EX[:, :, None].to_broadcast([128, T, 16]), op=ALU.mult)
                nc.vector.tensor_copy(out=Gt[:, :, 16], in_=EX[:])
                pM = ps.tile([128, 17], F32, tag="pacc_main", space="PSUM")
                for t in range(T):
                    nc.tensor.matmul(out=pM[:], lhsT=S[:, t, :], rhs=Gt[:, t, :17],
                                     start=(t == 0), stop=(t == T - 1))
                eSl = sb.tile([128, 1], BF16, tag="eSl")
                nc.vector.tensor_tensor(out=eSl[:], in0=zg[:, 16:17], in1=zg[:, 17:18], op=ALU.add)
                eSa = sb.tile([128, 1], BF16, tag="eSa")
                nc.vector.scalar_tensor_tensor(out=eSa[:], in0=eSl[:], scalar=0.2,
                                               in1=eSl[:], op0=ALU.mult, op1=ALU.max)
                exS = sb.tile([128, 1], BF16, tag="exS")
                nc.scalar.activation(out=exS[:], in_=eSa[:], func=ACT.Exp)
                selfz = sb.tile([128, 16], BF16, tag="selfz")
                nc.vector.tensor_tensor(out=selfz[:], in0=zg[:, :16],
                                        in1=exS[:, :1].to_broadcast([128, 16]), op=ALU.mult)
                selfc = sb.tile([128, 16], F32, tag="selfc")
                nc.vector.tensor_tensor(out=selfc[:], in0=pM[:, :16], in1=selfz[:], op=ALU.add)
                sS = sb.tile([128, 1], F32, tag="sS")
                nc.vector.tensor_tensor(out=sS[:], in0=pM[:, 16:17], in1=exS[:], op=ALU.add)
                rS = sb.tile([128, 1], F32, tag="rS")
                nc.vector.reciprocal(out=rS[:], in_=sS[:])
                nc.vector.tensor_scalar(out=selfc[:], in0=selfc[:], scalar1=rS[:, :1],
                                        scalar2=None, op0=ALU.mult)
                nc.vector.tensor_tensor(out=selfc[:], in0=selfc[:], in1=C["b3t"][:], op=ALU.add)
                h3 = layer_norm_elu(nc, sb, selfc, C["g3t"], C["be3t"], 16, epsc)
                OB = sb.tile([128, G], BF16, tag="OB")
                nc.vector.tensor_tensor(
                    out=OB[:], in0=C["iotaF"][:, :G],
                    in1=bg[:, :1].to_broadcast([128, G]), op=ALU.is_equal)
                h3w = sb.tile([128, 17], BF16, tag="h3w")
                nc.vector.tensor_copy(out=h3w[:, :16], in_=h3[:])
                nc.vector.memset(h3w[:, 16:17], 1.0)
                nc.tensor.matmul(out=pPool[:G, :17], lhsT=OB[:], rhs=h3w[:],
                                 start=(g == 0), stop=(g == NG - 1))
            po = sb.tile([128, 17], F32, tag="po")
            if phases >= 3:
                nc.vector.tensor_copy(out=po[:G, :], in_=pPool[:G, :])
            else:
                nc.vector.memset(po[:, :], 0.0)
            nc.sync.dma_start(out=t_out[:, :], in_=po[:G, :])
    nc.finalize()
    return nc


_CACHE = {}


def kernel(**inputs):
    consts, percore, meta, host = host_prep(inputs)
    key = tuple(tuple(r) for r in meta['Tn'])
    if key not in _CACHE:
        _CACHE[key] = build(meta)
    nc = _CACHE[key]
    in_maps = []
    for c in range(NC_FULL):
        m = dict(consts)
        m.update(percore[c])
        in_maps.append(m)
    from concourse.bass_utils import run_bass_kernel_spmd
    res = run_bass_kernel_spmd(nc, in_maps, core_ids=list(range(NC_FULL)))
    parts = np.stack([r["part"] for r in res.results])
    tot = parts.sum(axis=0)
    pooled = tot[:, :16] / np.maximum(tot[:, 16:17], 1.0)
    h = np.maximum(pooled @ host['fcW1'] + host['fcb1'], 0.0)
    return (h @ host['fcW2'] + host['fcb2']).astype(np.float32)


# revision 15
# speedup vs baseline: 1.3801x; 1.0026x over previous
"""MinamoTopoModel GAT kernel: host preprocessing + Bass builder (v2, bf16).

Design (8-core SPMD, dst-sharded, bf16 records):
  L1: cnt-histogram trick -> stacked-head matmuls (2 transposes + 2 matmuls),
      LN+ELU, phase-A producing L2 records [z512|al4|ar4] (bf16, 640-elem
      1280B rows) written to 3 local shard tables + a compact attn table.
  Node shards split into 3 tables at group boundaries [0,32,46,49] so each
      table gets ONE AllGather (Shared single-writer) that can start before
      L1 finishes, and every table has <=32768 rows (int16 dma_gather idx).
  L2/L3: per-group batched dma_gather of src records (one per table) +
      batched dst-attn dma_gather from local tables, segment softmax without
      max-subtraction, S-matrix (iota compare) PSUM scatter matmuls,
      self-loops handled per-group directly.
  Graph pooling -> per-core [50,17] partials; final FC on host.
"""
import numpy as np
import ml_dtypes
import concourse.bacc as bacc
import concourse.bass as bass
import concourse.mybir as mybir
import concourse.tile as tile

F32 = mybir.dt.float32
BF16 = mybir.dt.bfloat16
I16 = mybir.dt.int16
AX = mybir.AxisListType
ALU = mybir.AluOpType
ACT = mybir.ActivationFunctionType
EPS = 1e-5
BF = ml_dtypes.bfloat16

N_FULL, E_FULL, G_FULL, NC_FULL = 50000, 800000, 50, 8
NPC = N_FULL // NC_FULL            # 6250
NG = (NPC + 127) // 128            # 49
NPCP = NG * 128                    # 6272
TBOUND = [0, 32, 46, 49]           # table split points (groups)
NTAB = 3
TLO = [b * 128 for b in TBOUND[:-1]]            # local row starts
TSPAN = [(TBOUND[i + 1] - TBOUND[i]) * 128 for i in range(NTAB)]   # 4096,1792,384
REC2 = 640                         # bf16: z512 al4 ar4 pad -> 1280B rows
REC3 = 128                         # bf16: z16 al ar pad -> 256B rows
MAXT = 8                           # tiles per dma_gather (1024-idx HW limit)


def _wrap_idx(flat):
    """softdge idx wrap: flat slot i -> partition i%16, col i//16; x8 copies."""
    n = len(flat)
    assert n % 16 == 0
    w = np.ascontiguousarray(flat.reshape(n // 16, 16).T.astype(np.int16))
    return np.tile(w, (8, 1))


def host_prep(inputs, TILE=32, EMB=16):
    NC = NC_FULL
    H1, C1, H2, C2, H3, C3 = 8, 64, 4, 128, 1, 16
    x = np.asarray(inputs['x']).astype(np.int64)
    ei = np.asarray(inputs['edge_index']).astype(np.int64)
    batch = np.asarray(inputs['batch']).astype(np.int64)
    emb = np.asarray(inputs['emb'], np.float32)
    W1 = np.asarray(inputs['W1'], np.float32)
    as1 = np.asarray(inputs['a_src1'], np.float32); ad1 = np.asarray(inputs['a_dst1'], np.float32)
    b1 = np.asarray(inputs['b1'], np.float32)
    g1 = np.asarray(inputs['g1'], np.float32); be1 = np.asarray(inputs['be1'], np.float32)
    W2 = np.asarray(inputs['W2'], np.float32)
    as2 = np.asarray(inputs['a_src2'], np.float32); ad2 = np.asarray(inputs['a_dst2'], np.float32)
    b2 = np.asarray(inputs['b2'], np.float32)
    g2 = np.asarray(inputs['g2'], np.float32); be2 = np.asarray(inputs['be2'], np.float32)
    W3 = np.asarray(inputs['W3'], np.float32)
    as3 = np.asarray(inputs['a_src3'], np.float32); ad3 = np.asarray(inputs['a_dst3'], np.float32)
    b3 = np.asarray(inputs['b3'], np.float32)
    g3 = np.asarray(inputs['g3'], np.float32); be3 = np.asarray(inputs['be3'], np.float32)

    # ---- L1 tables (cnt trick) ----
    z1 = emb @ W1                                     # [32, 512]
    z1h = z1.reshape(TILE, H1, C1)
    al1t = np.einsum('thc,hc->th', z1h, as1)          # [32,8]
    ar1t = np.einsum('thc,hc->th', z1h, ad1)
    ee = al1t.T[None, :, :] + ar1t[:, :, None]        # [xd=32, h=8, t=32]
    ee = np.where(ee > 0, ee, 0.2 * ee)
    E_tab = np.exp(ee).astype(np.float32)             # [32, 8, 32]

    src_all = np.concatenate([ei[0], np.arange(N_FULL)])
    dst_all = np.concatenate([ei[1], np.arange(N_FULL)])
    xs_all = x[src_all]
    cnt = np.zeros((N_FULL, TILE), np.float32)
    np.add.at(cnt, (dst_all, xs_all), 1.0)

    # ---- Z1 stacked-head tables: Z1A/Z1B [128, 256] ----
    z1b = z1.astype(BF).astype(np.float32)
    Z1A = np.zeros((128, 256), np.float32)
    Z1B = np.zeros((128, 256), np.float32)
    for h in range(4):
        Z1A[h * 32:(h + 1) * 32, h * 64:(h + 1) * 64] = z1b[:, h * 64:(h + 1) * 64]
        Z1B[h * 32:(h + 1) * 32, h * 64:(h + 1) * 64] = z1b[:, (h + 4) * 64:(h + 5) * 64]

    # ---- weight tables W' = [W | W@As | W@Ad] ----
    def wprime(W, a_s, a_d, H, C):
        As = np.zeros((H * C, H), np.float32)
        Ad = np.zeros((H * C, H), np.float32)
        for h in range(H):
            As[h * C:(h + 1) * C, h] = a_s[h]
            Ad[h * C:(h + 1) * C, h] = a_d[h]
        return np.concatenate([W, W @ As, W @ Ad], axis=1)

    W2p = wprime(W2, as2, ad2, H2, C2)                # [512, 520]
    W3p = wprime(W3, as3, ad3, H3, C3)                # [512, 18]
    W2c = np.ascontiguousarray(W2p.reshape(4, 128, 520))
    W3c = np.ascontiguousarray(W3p.reshape(4, 128, 18))

    def bc(v, F):
        t = np.zeros((128, F), np.float32); t[:, :] = v[None, :F]; return t

    consts = dict(
        W2c=W2c.astype(BF), W3c=W3c.astype(BF),
        Z1A=Z1A.astype(BF), Z1B=Z1B.astype(BF),
        b1t=bc(b1, 512), g1t=bc(g1, 512), be1t=bc(be1, 512),
        b2t=bc(b2, 512), g2t=bc(g2, 512), be2t=bc(be2, 512),
        b3t=bc(b3, 16), g3t=bc(g3, 16), be3t=bc(be3, 16),
        iotaF=np.tile(np.arange(128, dtype=np.float32), (128, 1)).astype(BF),
        ident=np.eye(128, dtype=np.float32).astype(BF),
    )

    # ---- per-core edge bucketing (non-self edges only) ----
    es, ed = ei[0], ei[1]
    core_of = ed // NPC
    dr = ed % NPC
    grp_of = dr // 128
    dloc = dr % 128
    sc = es // NPC
    sr = es % NPC
    ti = np.searchsorted(np.array(TLO[1:]), sr, side='right')     # table id 0..2
    tlo = np.array(TLO)[ti]
    gidx = sc * np.array(TSPAN)[ti] + (sr - tlo)                  # row in table

    order = np.lexsort((gidx, ti, grp_of, core_of))
    core_s = core_of[order]; grp_s = grp_of[order]
    ti_s = ti[order]; gidx_s = gidx[order]
    dloc_s = dloc[order]

    key = (core_s * NG + grp_s) * NTAB + ti_s
    bounds = np.searchsorted(key, np.arange(NC * NG * NTAB + 1))
    cntT = (bounds[1:] - bounds[:-1]).reshape(NC, NG, NTAB)
    Tn = np.maximum(1, -(-cntT.max(axis=0) // 128))               # [NG, NTAB]
    Tt = Tn.sum(axis=1)                                           # [NG]
    oN = np.zeros((NTAB, NG + 1), np.int64)
    for t in range(NTAB):
        oN[t, 1:] = np.cumsum(Tn[:, t])
    oT = np.concatenate([[0], np.cumsum(Tt)]).astype(np.int64)
    NTn = [int(oN[t, -1]) for t in range(NTAB)]
    NTT = int(oT[-1])

    percore = []
    for c in range(NC):
        idxN = [np.zeros((128, NTn[t] * 8), np.int16) for t in range(NTAB)]
        idxD = np.zeros((128, NTT * 8), np.int16)
        dlS = np.full((NTT, 128), 200.0, np.float32)
        for g in range(NG):
            r0 = g * 128
            tb = int(oT[g])
            for t in range(NTAB):
                k = (c * NG + g) * NTAB + t
                s, e = bounds[k], bounds[k + 1]
                n = e - s
                cap = int(Tn[g, t]) * 128
                assert n <= cap
                flat = np.zeros(cap, np.int64)
                flat[:n] = gidx_s[s:e]
                dl = np.full(cap, 200.0, np.float32)
                dl[:n] = dloc_s[s:e]
                o = int(oN[t, g])
                idxN[t][:, o * 8:(o + int(Tn[g, t])) * 8] = _wrap_idx(flat)
                dlS[tb:tb + int(Tn[g, t])] = dl.reshape(int(Tn[g, t]), 128)
                dfl = np.full(cap, float(r0), np.float32)
                dfl[:n] = r0 + dloc_s[s:e]
                idxD[:, tb * 8:(tb + int(Tn[g, t])) * 8] = _wrap_idx(dfl.astype(np.int64))
                tb += int(Tn[g, t])
        lo, hi = c * NPC, (c + 1) * NPC
        cntc = np.zeros((NPCP, TILE), np.float32)
        cntc[:NPC] = cnt[lo:hi]
        cntc[NPC:, 0] = 1.0
        Ec = np.zeros((NPCP, H1 * TILE), np.float32)
        Ec[:NPC] = E_tab[x[lo:hi]].reshape(NPC, H1 * TILE)
        Ec[NPC:] = 1.0
        batchc = np.full((NPCP, 1), 200.0, np.float32)
        batchc[:NPC, 0] = batch[lo:hi]
        percore.append(dict(
            cntc=cntc.astype(BF), Ec=Ec.astype(BF),
            batchc=batchc,
            idx0=idxN[0], idx1=idxN[1], idx2=idxN[2], idxD=idxD,
            dlS=np.ascontiguousarray(dlS.T).astype(BF),   # [128, NTT]
        ))

    meta = dict(Tn=Tn.tolist(), Tt=Tt.tolist(),
                oN=oN.tolist(), oT=oT.tolist(),
                NTn=NTn, NTT=NTT)
    host = dict(fcW1=np.asarray(inputs['fcW1'], np.float32),
                fcb1=np.asarray(inputs['fcb1'], np.float32),
                fcW2=np.asarray(inputs['fcW2'], np.float32),
                fcb2=np.asarray(inputs['fcb2'], np.float32))
    return consts, percore, meta, host


def layer_norm_elu(nc, pool, y, g_t, be_t, F, epsc):
    """In SBUF: y f32 [128,F] -> elu(LN(y)*g+be) f32. Returns new tile."""
    s1 = pool.tile([128, 1], F32, tag="ln_s1")
    nc.vector.tensor_reduce(out=s1[:], in_=y[:], axis=AX.X, op=ALU.add)
    m2 = pool.tile([128, 1], F32, tag="ln_m2")
    nc.vector.tensor_scalar_mul(out=m2[:], in0=s1[:], scalar1=-1.0 / F)
    sq = pool.tile([128, F], F32, tag="ln_sq")
    ss = pool.tile([128, 1], F32, tag="ln_ss")
    nc.scalar.activation(out=sq[:], in_=y[:], func=ACT.Square, bias=m2[:, :1],
                         accum_out=ss[:])
    sd = pool.tile([128, 1], F32, tag="ln_sd")
    nc.scalar.activation(out=sd[:], in_=ss[:], func=ACT.Sqrt, bias=epsc[:, :1], scale=1.0 / F)
    rs = pool.tile([128, 1], F32, tag="ln_rs")
    nc.vector.reciprocal(out=rs[:], in_=sd[:])
    nc.vector.tensor_scalar(out=y[:], in0=y[:], scalar1=m2[:, :1], scalar2=rs[:, :1],
                            op0=ALU.add, op1=ALU.mult)
    nc.vector.tensor_tensor(out=y[:], in0=y[:], in1=g_t[:, :F], op=ALU.mult)
    nc.vector.tensor_tensor(out=y[:], in0=y[:], in1=be_t[:, :F], op=ALU.add)
    # ELU = max(x,0) + exp(min(x,0)) - 1
    nc.vector.tensor_scalar_min(out=sq[:], in0=y[:], scalar1=0.0)
    nc.scalar.activation(out=sq[:], in_=sq[:], func=ACT.Exp)
    h = pool.tile([128, F], F32, tag="elu_h")
    nc.vector.tensor_scalar(out=h[:], in0=y[:], scalar1=0.0, scalar2=-1.0,
                            op0=ALU.max, op1=ALU.add)
    nc.vector.tensor_tensor(out=h[:], in0=h[:], in1=sq[:], op=ALU.add)
    return h


def build(meta, phases=3):
    Tn, Tt = meta['Tn'], meta['Tt']
    oN, oT = meta['oN'], meta['oT']
    NTn, NTT = meta['NTn'], meta['NTT']
    NC, G = NC_FULL, G_FULL
    TILE, H1 = 32, 8
    H2x = 4

    def tab_of(g):
        for t in range(NTAB):
            if g < TBOUND[t + 1]:
                return t, (g - TBOUND[t]) * 128
        raise AssertionError

    nc = bacc.Bacc("TRN2", num_devices=NC, num_swdge_queues=4)
    t_cnt = nc.dram_tensor("cntc", [NPCP, TILE], BF16, kind="ExternalInput")
    t_E = nc.dram_tensor("Ec", [NPCP, H1 * TILE], BF16, kind="ExternalInput")
    t_bat = nc.dram_tensor("batchc", [NPCP, 1], F32, kind="ExternalInput")
    t_iN = [nc.dram_tensor(f"idx{t}", [128, NTn[t] * 8], I16, kind="ExternalInput")
            for t in range(NTAB)]
    t_iD = nc.dram_tensor("idxD", [128, NTT * 8], I16, kind="ExternalInput")
    t_dl = nc.dram_tensor("dlS", [128, NTT], BF16, kind="ExternalInput")
    t_W2c = nc.dram_tensor("W2c", [4, 128, 520], BF16, kind="ExternalInput")
    t_W3c = nc.dram_tensor("W3c", [4, 128, 18], BF16, kind="ExternalInput")
    t_Z1A = nc.dram_tensor("Z1A", [128, 256], BF16, kind="ExternalInput")
    t_Z1B = nc.dram_tensor("Z1B", [128, 256], BF16, kind="ExternalInput")
    cn = {}
    for nm, sh in [("b1t", 512), ("g1t", 512), ("be1t", 512), ("b2t", 512),
                   ("g2t", 512), ("be2t", 512), ("b3t", 16), ("g3t", 16), ("be3t", 16)]:
        cn[nm] = nc.dram_tensor(nm, [128, sh], F32, kind="ExternalInput")
    t_iota = nc.dram_tensor("iotaF", [128, 128], BF16, kind="ExternalInput")
    t_id = nc.dram_tensor("ident", [128, 128], BF16, kind="ExternalInput")
    t_out = nc.dram_tensor("part", [G, 17], F32, kind="ExternalOutput")

    with tile.TileContext(nc) as tc:
        with tc.tile_pool(name="const", bufs=1) as cp, \
             tc.tile_pool(name="sb", bufs=2) as sb, \
             tc.tile_pool(name="gbuf", bufs=2) as gb, \
             tc.tile_pool(name="dbuf", bufs=2) as db, \
             tc.tile_pool(name="ps", bufs=1, space="PSUM") as ps, \
             tc.tile_pool(name="pst", bufs=2, space="PSUM") as pst, \
             tc.tile_pool(name="pacc", bufs=1, space="PSUM") as pacc, \
             tc.tile_pool(name="dram", bufs=1, space="DRAM") as dp:

            # ---- const loads ----
            C = {}
            for nm, src, shp in [("iotaF", t_iota, [128, 128]), ("ident", t_id, [128, 128]),
                                 ("Z1A", t_Z1A, [128, 256]), ("Z1B", t_Z1B, [128, 256])]:
                C[nm] = cp.tile(shp, BF16, tag="c_" + nm, name="c_" + nm)
                nc.sync.dma_start(out=C[nm][:], in_=src[:])
            for nm in cn:
                F = 512 if nm[-2] != '3' else 16
                C[nm] = cp.tile([128, F], F32, tag="c_" + nm, name="c_" + nm)
                nc.sync.dma_start(out=C[nm][:], in_=cn[nm][:])
            W2s = cp.tile([128, 4 * 520], BF16, name="W2s")
            nc.sync.dma_start(out=W2s[:].rearrange("p (a b) -> p a b", a=4),
                              in_=t_W2c[:].rearrange("a p b -> p a b"))
            W3s = cp.tile([128, 4 * 18], BF16, name="W3s")
            nc.sync.dma_start(out=W3s[:].rearrange("p (a b) -> p a b", a=4),
                              in_=t_W3c[:].rearrange("a p b -> p a b"))
            epsc = cp.tile([128, 1], F32, name="epsc")
            nc.vector.memset(epsc[:], EPS)

            def gq(out_ap, in_ap, idxs_ap, ni, elem):
                nc.gpsimd.dma_gather(
                    out_ap=out_ap, in_ap=in_ap, idxs_ap=idxs_ap,
                    num_idxs=ni, num_idxs_reg=ni, elem_size=elem,
                    single_packet=False)

            def fire():
                pass

            def await_gathers():
                pass

            IN = []
            for t in range(NTAB):
                it = cp.tile([128, NTn[t] * 8], I16, name=f"c_idx{t}")
                nc.sync.dma_start(out=it[:], in_=t_iN[t][:])
                IN.append(it)
            ID = cp.tile([128, NTT * 8], I16, name="c_idxD")
            nc.sync.dma_start(out=ID[:], in_=t_iD[:])
            DL = cp.tile([128, NTT], BF16, name="c_dl")
            nc.sync.dma_start(out=DL[:], in_=t_dl[:])

            rec2_sh = [dp.tile([TSPAN[t], REC2], BF16, name=f"rec2_sh{t}")
                       for t in range(NTAB)]
            rec2_full = [dp.tile([NC * TSPAN[t], REC2], BF16, addr_space="Shared",
                                 name=f"rec2_full{t}") for t in range(NTAB)]
            att2 = dp.tile([NPCP, REC3], BF16, name="att2")
            rec3_sh = [dp.tile([TSPAN[t], REC3], BF16, name=f"rec3_sh{t}")
                       for t in range(NTAB)]
            rec3_full = [dp.tile([NC * TSPAN[t], REC3], BF16, addr_space="Shared",
                                 name=f"rec3_full{t}") for t in range(NTAB)]
            att3 = dp.tile([NPCP, REC3], BF16, name="att3")

            # ================= L1 + phaseA(L2) =================
            for g in range(NG):
                r0 = g * 128
                tb, rk = tab_of(g)
                cg = sb.tile([128, TILE], BF16, tag="cg")
                nc.sync.dma_start(out=cg[:], in_=t_cnt[r0:r0 + 128, :])
                Eg = sb.tile([128, H1, TILE], BF16, tag="Eg")
                nc.sync.dma_start(out=Eg[:, :, :],
                                  in_=t_E[r0:r0 + 128, :].rearrange("p (h t) -> p h t", h=H1))
                M = sb.tile([128, H1, TILE], BF16, tag="M")
                nc.vector.tensor_tensor(out=M[:, :, :], in0=Eg[:, :, :],
                                        in1=cg[:, None, :].to_broadcast([128, H1, TILE]),
                                        op=ALU.mult)
                s = sb.tile([128, H1], F32, tag="s")
                nc.vector.tensor_reduce(out=s[:], in_=M[:, :, :], axis=AX.X, op=ALU.add)
                rs = sb.tile([128, H1], F32, tag="rs")
                nc.vector.reciprocal(out=rs[:], in_=s[:])
                P = sb.tile([128, H1, TILE], BF16, tag="P")
                nc.vector.tensor_tensor(out=P[:, :, :], in0=M[:, :, :],
                                        in1=rs[:, :, None].to_broadcast([128, H1, TILE]),
                                        op=ALU.mult)
                pO = ps.tile([128, 512], F32, tag="pacc_main", space="PSUM")
                for half in range(2):
                    ptp = pst.tile([128, 128], BF16, tag="tp_ps", space="PSUM")
                    nc.tensor.transpose(
                        out=ptp[:],
                        in_=P[:, half * 4:(half + 1) * 4, :].rearrange("p h t -> p (h t)"),
                        identity=C["ident"][:])
                    PT = sb.tile([128, 128], BF16, tag="PT")
                    nc.vector.tensor_copy(out=PT[:], in_=ptp[:])
                    nc.tensor.matmul(out=pO[:, half * 256:(half + 1) * 256], lhsT=PT[:],
                                     rhs=C["Z1A" if half == 0 else "Z1B"][:],
                                     start=True, stop=True)
                y = sb.tile([128, 512], F32, tag="y1")
                nc.vector.tensor_tensor(out=y[:], in0=pO[:], in1=C["b1t"][:], op=ALU.add)
                h1 = layer_norm_elu(nc, sb, y, C["g1t"], C["be1t"], 512, epsc)
                h1b = sb.tile([128, 512], BF16, tag="h1b")
                nc.vector.tensor_copy(out=h1b[:], in_=h1[:])
                z2p = ps.tile([128, 512], F32, tag="pz", space="PSUM")
                z2pb = ps.tile([128, 8], F32, tag="pzb", space="PSUM")
                for q in range(4):
                    ptp = pst.tile([128, 128], BF16, tag="tp_ps", space="PSUM")
                    nc.tensor.transpose(out=ptp[:], in_=h1b[:, q * 128:(q + 1) * 128],
                                        identity=C["ident"][:])
                    hT = sb.tile([128, 128], BF16, tag="hT")
                    nc.vector.tensor_copy(out=hT[:], in_=ptp[:])
                    nc.tensor.matmul(out=z2p[:], lhsT=hT[:], rhs=W2s[:, q * 520:q * 520 + 512],
                                     start=(q == 0), stop=(q == 3))
                    nc.tensor.matmul(out=z2pb[:], lhsT=hT[:], rhs=W2s[:, q * 520 + 512:(q + 1) * 520],
                                     start=(q == 0), stop=(q == 3))
                zs = sb.tile([128, REC2], BF16, tag="zs")
                nc.vector.memset(zs[:, 520:], 0.0)
                nc.vector.tensor_copy(out=zs[:, :512], in_=z2p[:])
                nc.vector.tensor_copy(out=zs[:, 512:520], in_=z2pb[:])
                nc.sync.dma_start(out=rec2_sh[tb][rk:rk + 128, :], in_=zs[:])
                nc.sync.dma_start(out=att2[r0:r0 + 128, :], in_=zs[:, 512:640])
                if g + 1 in TBOUND:
                    nc.gpsimd.collective_compute(
                        "AllGather", ALU.bypass, replica_groups=[list(range(NC))],
                        ins=[rec2_sh[tb].opt()], outs=[rec2_full[tb].opt()])

            # ================= L2 + phaseA(L3) =================
            for g in range(NG if phases >= 2 else 0):
                r0 = g * 128
                tb, rk = tab_of(g)
                T = Tt[g]
                o0 = oT[g]
                dl = DL[:, o0:o0 + T]
                zg = sb.tile([128, 520], BF16, tag="zg")
                nc.sync.dma_start(out=zg[:], in_=rec2_sh[tb][rk:rk + 128, 0:520])
                Gt = gb.tile([128, T, REC2], BF16, tag="G")
                tacc = 0
                for t in range(NTAB):
                    for c0 in range(0, Tn[g][t], MAXT):
                        cw = min(MAXT, Tn[g][t] - c0)
                        gq(Gt[:, tacc + c0:tacc + c0 + cw, :], rec2_full[t][:],
                           IN[t][:, (oN[t][g] + c0) * 8:(oN[t][g] + c0 + cw) * 8],
                           cw * 128, REC2)
                    tacc += Tn[g][t]
                fire()
                S = gb.tile([128, T, 128], BF16, tag="S")
                nc.vector.tensor_tensor(
                    out=S[:, :, :],
                    in0=C["iotaF"][:, None, :].to_broadcast([128, T, 128]),
                    in1=dl[:, :, None].to_broadcast([128, T, 128]),
                    op=ALU.is_equal)
                pAR = ps.tile([128, T * H2x], F32, tag="pAR", space="PSUM")
                for t in range(T):
                    ptp = pst.tile([128, 128], BF16, tag="tp_ps", space="PSUM")
                    nc.tensor.transpose(out=ptp[:], in_=S[:, t, :], identity=C["ident"][:])
                    STt = sb.tile([128, 128], BF16, tag="STt")
                    nc.vector.tensor_copy(out=STt[:], in_=ptp[:])
                    nc.tensor.matmul(out=pAR[:, t * H2x:(t + 1) * H2x], lhsT=STt[:],
                                     rhs=zg[:, 516:520], start=True, stop=True)
                eL = sb.tile([128, T, H2x], BF16, tag="eL")
                nc.vector.tensor_tensor(
                    out=eL[:, :, :], in0=Gt[:, :, 512:516],
                    in1=pAR[:].rearrange("p (t h) -> p t h", h=H2x),
                    op=ALU.add)
                eA = sb.tile([128, T * H2x], BF16, tag="eA")
                nc.vector.scalar_tensor_tensor(
                    out=eA[:], in0=eL[:, :, :].rearrange("p t h -> p (t h)"),
                    scalar=0.2, in1=eL[:, :, :].rearrange("p t h -> p (t h)"),
                    op0=ALU.mult, op1=ALU.max)
                EX = sb.tile([128, T * H2x], BF16, tag="EX")
                nc.scalar.activation(out=EX[:], in_=eA[:], func=ACT.Exp)
                nc.vector.tensor_tensor(
                    out=Gt[:, :, :512].rearrange("p t (h c) -> p t h c", h=H2x),
                    in0=Gt[:, :, :512].rearrange("p t (h c) -> p t h c", h=H2x),
                    in1=EX[:].rearrange("p (t h) -> p t h", h=H2x)[:, :, :, None]
                        .to_broadcast([128, T, H2x, 128]),
                    op=ALU.mult)
                pMain = ps.tile([128, 512], F32, tag="pacc_main", space="PSUM")
                pS = ps.tile([128, H2x], F32, tag="pacc_s", space="PSUM")
                for t in range(T):
                    nc.tensor.matmul(out=pMain[:], lhsT=S[:, t, :], rhs=Gt[:, t, :512],
                                     start=(t == 0), stop=(t == T - 1))
                    nc.tensor.matmul(out=pS[:], lhsT=S[:, t, :], rhs=EX[:, t * H2x:(t + 1) * H2x],
                                     start=(t == 0), stop=(t == T - 1))
                # self-loop
                eSl = sb.tile([128, H2x], BF16, tag="eSl")
                nc.vector.tensor_tensor(out=eSl[:], in0=zg[:, 512:516], in1=zg[:, 516:520], op=ALU.add)
                eSa = sb.tile([128, H2x], BF16, tag="eSa")
                nc.vector.scalar_tensor_tensor(out=eSa[:], in0=eSl[:], scalar=0.2,
                                               in1=eSl[:], op0=ALU.mult, op1=ALU.max)
                exS = sb.tile([128, H2x], BF16, tag="exS")
                nc.scalar.activation(out=exS[:], in_=eSa[:], func=ACT.Exp)
                selfz = sb.tile([128, 512], BF16, tag="selfz")
                nc.vector.tensor_tensor(
                    out=selfz[:].rearrange("p (h c) -> p h c", h=H2x),
                    in0=zg[:, :512].rearrange("p (h c) -> p h c", h=H2x),
                    in1=exS[:, :, None].to_broadcast([128, H2x, 128]), op=ALU.mult)
                selfc = sb.tile([128, 512], F32, tag="selfc")
                nc.vector.tensor_tensor(out=selfc[:], in0=pMain[:], in1=selfz[:], op=ALU.add)
                sS = sb.tile([128, H2x], F32, tag="sS")
                nc.vector.tensor_tensor(out=sS[:], in0=pS[:], in1=exS[:], op=ALU.add)
                rS = sb.tile([128, H2x], F32, tag="rS")
                nc.vector.reciprocal(out=rS[:], in_=sS[:])
                nc.vector.tensor_tensor(
                    out=selfc[:].rearrange("p (h c) -> p h c", h=H2x),
                    in0=selfc[:].rearrange("p (h c) -> p h c", h=H2x),
                    in1=rS[:, :, None].to_broadcast([128, H2x, 128]), op=ALU.mult)
                nc.vector.tensor_tensor(out=selfc[:], in0=selfc[:], in1=C["b2t"][:], op=ALU.add)
                h2 = layer_norm_elu(nc, sb, selfc, C["g2t"], C["be2t"], 512, epsc)
                h2b = sb.tile([128, 512], BF16, tag="h2b")
                nc.vector.tensor_copy(out=h2b[:], in_=h2[:])
                z3p = ps.tile([128, 18], F32, tag="pz", space="PSUM")
                for q in range(4):
                    ptp = pst.tile([128, 128], BF16, tag="tp_ps", space="PSUM")
                    nc.tensor.transpose(out=ptp[:], in_=h2b[:, q * 128:(q + 1) * 128],
                                        identity=C["ident"][:])
                    hT = sb.tile([128, 128], BF16, tag="hT")
                    nc.vector.tensor_copy(out=hT[:], in_=ptp[:])
                    nc.tensor.matmul(out=z3p[:], lhsT=hT[:], rhs=W3s[:, q * 18:(q + 1) * 18],
                                     start=(q == 0), stop=(q == 3))
                z3s = sb.tile([128, REC3], BF16, tag="z3s")
                nc.vector.memset(z3s[:, 18:], 0.0)
                nc.vector.tensor_copy(out=z3s[:, :18], in_=z3p[:])
                nc.sync.dma_start(out=rec3_sh[tb][rk:rk + 128, :], in_=z3s[:])
                nc.sync.dma_start(out=att3[r0:r0 + 128, :], in_=z3s[:])
                if g + 1 in TBOUND:
                    nc.gpsimd.collective_compute(
                        "AllGather", ALU.bypass, replica_groups=[list(range(NC))],
                        ins=[rec3_sh[tb].opt()], outs=[rec3_full[tb].opt()])

            # ================= L3 + pooling =================
            pPool = pacc.tile([128, 17], F32, tag="pPool", space="PSUM")
            for g in range(NG if phases >= 3 else 0):
                r0 = g * 128
                tb, rk = tab_of(g)
                T = Tt[g]
                o0 = oT[g]
                dl = DL[:, o0:o0 + T]
                zg = sb.tile([128, 18], BF16, tag="zg")
                nc.sync.dma_start(out=zg[:], in_=rec3_sh[tb][rk:rk + 128, 0:18])
                bg = sb.tile([128, 1], F32, tag="bg")
                nc.sync.dma_start(out=bg[:], in_=t_bat[r0:r0 + 128, :])
                Gt = gb.tile([128, T, REC3], BF16, tag="G")
                tacc = 0
                for t in range(NTAB):
                    for c0 in range(0, Tn[g][t], MAXT):
                        cw = min(MAXT, Tn[g][t] - c0)
                        gq(Gt[:, tacc + c0:tacc + c0 + cw, :], rec3_full[t][:],
                           IN[t][:, (oN[t][g] + c0) * 8:(oN[t][g] + c0 + cw) * 8],
                           cw * 128, REC3)
                    tacc += Tn[g][t]
                fire()
                S = gb.tile([128, T, 128], BF16, tag="S")
                nc.vector.tensor_tensor(
                    out=S[:, :, :],
                    in0=C["iotaF"][:, None, :].to_broadcast([128, T, 128]),
                    in1=dl[:, :, None].to_broadcast([128, T, 128]),
                    op=ALU.is_equal)
                pAR = ps.tile([128, T], F32, tag="pAR", space="PSUM")
                for t in range(T):
                    ptp = pst.tile([128, 128], BF16, tag="tp_ps", space="PSUM")
                    nc.tensor.transpose(out=ptp[:], in_=S[:, t, :], identity=C["ident"][:])
                    STt = sb.tile([128, 128], BF16, tag="STt")
                    nc.vector.tensor_copy(out=STt[:], in_=ptp[:])
                    nc.tensor.matmul(out=pAR[:, t:t + 1], lhsT=STt[:],
                                     rhs=zg[:, 17:18], start=True, stop=True)
                eL = sb.tile([128, T], BF16, tag="eL")
                nc.vector.tensor_tensor(out=eL[:], in0=Gt[:, :, 16],
                                        in1=pAR[:],
                                        op=ALU.add)
                eA = sb.tile([128, T], BF16, tag="eA")
                nc.vector.scalar_tensor_tensor(out=eA[:], in0=eL[:], scalar=0.2,
                                               in1=eL[:], op0=ALU.mult, op1=ALU.max)
                EX = sb.tile([128, T], BF16, tag="EX")
                nc.scalar.activation(out=EX[:], in_=eA[:], func=ACT.Exp)
                nc.vector.tensor_tensor(
                    out=Gt[:, :, :16], in0=Gt[:, :, :16],
                    in1=EX[:, :, None].to_broadcast([128, T, 16]), op=ALU.mult)
                nc.vector.tensor_copy(out=Gt[:, :, 16], in_=EX[:])
                pM = ps.tile([128, 17], F32, tag="pacc_main", space="PSUM")
                for t in range(T):
                    nc.tensor.matmul(out=pM[:], lhsT=S[:, t, :], rhs=Gt[:, t, :17],
                                     start=(t == 0), stop=(t == T - 1))
                eSl = sb.tile([128, 1], BF16, tag="eSl")
                nc.vector.tensor_tensor(out=eSl[:], in0=zg[:, 16:17], in1=zg[:, 17:18], op=ALU.add)
                eSa = sb.tile([128, 1], BF16, tag="eSa")
                nc.vector.scalar_tensor_tensor(out=eSa[:], in0=eSl[:], scalar=0.2,
                                               in1=eSl[:], op0=ALU.mult, op1=ALU.max)
                exS = sb.tile([128, 1], BF16, tag="exS")
                nc.scalar.activation(out=exS[:], in_=eSa[:], func=ACT.Exp)
                selfz = sb.tile([128, 16], BF16, tag="selfz")
                nc.vector.tensor_tensor(out=selfz[:], in0=zg[:, :16],
                                        in1=exS[:, :1].to_broadcast([128, 16]), op=ALU.mult)
                selfc = sb.tile([128, 16], F32, tag="selfc")
                nc.vector.tensor_tensor(out=selfc[:], in0=pM[:, :16], in1=selfz[:], op=ALU.add)
                sS = sb.tile([128, 1], F32, tag="sS")
                nc.vector.tensor_tensor(out=sS[:], in0=pM[:, 16:17], in1=exS[:], op=ALU.add)
                rS = sb.tile([128, 1], F32, tag="rS")
                nc.vector.reciprocal(out=rS[:], in_=sS[:])
                nc.vector.tensor_scalar(out=selfc[:], in0=selfc[:], scalar1=rS[:, :1],
                                        scalar2=None, op0=ALU.mult)
                nc.vector.tensor_tensor(out=selfc[:], in0=selfc[:], in1=C["b3t"][:], op=ALU.add)
                h3 = layer_norm_elu(nc, sb, selfc, C["g3t"], C["be3t"], 16, epsc)
                OB = sb.tile([128, G], BF16, tag="OB")
                nc.vector.tensor_tensor(
                    out=OB[:], in0=C["iotaF"][:, :G],
                    in1=bg[:, :1].to_broadcast([128, G]), op=ALU.is_equal)
                h3w = sb.tile([128, 17], BF16, tag="h3w")
                nc.vector.tensor_copy(out=h3w[:, :16], in_=h3[:])
                nc.vector.memset(h3w[:, 16:17], 1.0)
                nc.tensor.matmul(out=pPool[:G, :17], lhsT=OB[:], rhs=h3w[:],
                                 start=(g == 0), stop=(g == NG - 1))
            po = sb.tile([128, 17], F32, tag="po")
            if phases >= 3:
                nc.vector.tensor_copy(out=po[:G, :], in_=pPool[:G, :])
            else:
                nc.vector.memset(po[:, :], 0.0)
            nc.sync.dma_start(out=t_out[:, :], in_=po[:G, :])
    nc.finalize()
    return nc


_CACHE = {}


def kernel(**inputs):
    consts, percore, meta, host = host_prep(inputs)
    key = tuple(tuple(r) for r in meta['Tn'])
    if key not in _CACHE:
        _CACHE[key] = build(meta)
    nc = _CACHE[key]
    in_maps = []
    for c in range(NC_FULL):
        m = dict(consts)
        m.update(percore[c])
        in_maps.append(m)
    from concourse.bass_utils import run_bass_kernel_spmd
    res = run_bass_kernel_spmd(nc, in_maps, core_ids=list(range(NC_FULL)))
    parts = np.stack([r["part"] for r in res.results])
    tot = parts.sum(axis=0)
    pooled = tot[:, :16] / np.maximum(tot[:, 16:17], 1.0)
    h = np.maximum(pooled @ host['fcW1'] + host['fcb1'], 0.0)
    return (h @ host['fcW2'] + host['fcb2']).astype(np.float32)


# revision 17
# speedup vs baseline: 1.3836x; 1.0026x over previous
"""MinamoTopoModel GAT kernel: host preprocessing + Bass builder (v2, bf16).

Design (8-core SPMD, dst-sharded, bf16 records):
  L1: cnt-histogram trick -> stacked-head matmuls (2 transposes + 2 matmuls),
      LN+ELU, phase-A producing L2 records [z512|al4|ar4] (bf16, 640-elem
      1280B rows) written to 3 local shard tables + a compact attn table.
  Node shards split into 3 tables at group boundaries [0,32,46,49] so each
      table gets ONE AllGather (Shared single-writer) that can start before
      L1 finishes, and every table has <=32768 rows (int16 dma_gather idx).
  L2/L3: per-group batched dma_gather of src records (one per table) +
      batched dst-attn dma_gather from local tables, segment softmax without
      max-subtraction, S-matrix (iota compare) PSUM scatter matmuls,
      self-loops handled per-group directly.
  Graph pooling -> per-core [50,17] partials; final FC on host.
"""
import numpy as np
import ml_dtypes
import concourse.bacc as bacc
import concourse.bass as bass
import concourse.mybir as mybir
import concourse.tile as tile

F32 = mybir.dt.float32
BF16 = mybir.dt.bfloat16
I16 = mybir.dt.int16
AX = mybir.AxisListType
ALU = mybir.AluOpType
ACT = mybir.ActivationFunctionType
EPS = 1e-5
BF = ml_dtypes.bfloat16

N_FULL, E_FULL, G_FULL, NC_FULL = 50000, 800000, 50, 8
NPC = N_FULL // NC_FULL            # 6250
NG = (NPC + 127) // 128            # 49
NPCP = NG * 128                    # 6272
TBOUND = [0, 32, 46, 49]           # table split points (groups)
NTAB = 3
TLO = [b * 128 for b in TBOUND[:-1]]            # local row starts
TSPAN = [(TBOUND[i + 1] - TBOUND[i]) * 128 for i in range(NTAB)]   # 4096,1792,384
REC2 = 640                         # bf16: z512 al4 ar4 pad -> 1280B rows
REC3 = 128                         # bf16: z16 al ar pad -> 256B rows
MAXT = 8                           # tiles per dma_gather (1024-idx HW limit)


def _wrap_idx(flat):
    """softdge idx wrap: flat slot i -> partition i%16, col i//16; x8 copies."""
    n = len(flat)
    assert n % 16 == 0
    w = np.ascontiguousarray(flat.reshape(n // 16, 16).T.astype(np.int16))
    return np.tile(w, (8, 1))


def host_prep(inputs, TILE=32, EMB=16):
    NC = NC_FULL
    H1, C1, H2, C2, H3, C3 = 8, 64, 4, 128, 1, 16
    x = np.asarray(inputs['x']).astype(np.int64)
    ei = np.asarray(inputs['edge_index']).astype(np.int64)
    batch = np.asarray(inputs['batch']).astype(np.int64)
    emb = np.asarray(inputs['emb'], np.float32)
    W1 = np.asarray(inputs['W1'], np.float32)
    as1 = np.asarray(inputs['a_src1'], np.float32); ad1 = np.asarray(inputs['a_dst1'], np.float32)
    b1 = np.asarray(inputs['b1'], np.float32)
    g1 = np.asarray(inputs['g1'], np.float32); be1 = np.asarray(inputs['be1'], np.float32)
    W2 = np.asarray(inputs['W2'], np.float32)
    as2 = np.asarray(inputs['a_src2'], np.float32); ad2 = np.asarray(inputs['a_dst2'], np.float32)
    b2 = np.asarray(inputs['b2'], np.float32)
    g2 = np.asarray(inputs['g2'], np.float32); be2 = np.asarray(inputs['be2'], np.float32)
    W3 = np.asarray(inputs['W3'], np.float32)
    as3 = np.asarray(inputs['a_src3'], np.float32); ad3 = np.asarray(inputs['a_dst3'], np.float32)
    b3 = np.asarray(inputs['b3'], np.float32)
    g3 = np.asarray(inputs['g3'], np.float32); be3 = np.asarray(inputs['be3'], np.float32)

    # ---- L1 tables (cnt trick) ----
    z1 = emb @ W1                                     # [32, 512]
    z1h = z1.reshape(TILE, H1, C1)
    al1t = np.einsum('thc,hc->th', z1h, as1)          # [32,8]
    ar1t = np.einsum('thc,hc->th', z1h, ad1)
    ee = al1t.T[None, :, :] + ar1t[:, :, None]        # [xd=32, h=8, t=32]
    ee = np.where(ee > 0, ee, 0.2 * ee)
    E_tab = np.exp(ee).astype(np.float32)             # [32, 8, 32]

    src_all = np.concatenate([ei[0], np.arange(N_FULL)])
    dst_all = np.concatenate([ei[1], np.arange(N_FULL)])
    xs_all = x[src_all]
    cnt = np.zeros((N_FULL, TILE), np.float32)
    np.add.at(cnt, (dst_all, xs_all), 1.0)

    # ---- Z1 stacked-head tables: Z1A/Z1B [128, 256] ----
    z1b = z1.astype(BF).astype(np.float32)
    Z1A = np.zeros((128, 256), np.float32)
    Z1B = np.zeros((128, 256), np.float32)
    for h in range(4):
        Z1A[h * 32:(h + 1) * 32, h * 64:(h + 1) * 64] = z1b[:, h * 64:(h + 1) * 64]
        Z1B[h * 32:(h + 1) * 32, h * 64:(h + 1) * 64] = z1b[:, (h + 4) * 64:(h + 5) * 64]

    # ---- weight tables W' = [W | W@As | W@Ad] ----
    def wprime(W, a_s, a_d, H, C):
        As = np.zeros((H * C, H), np.float32)
        Ad = np.zeros((H * C, H), np.float32)
        for h in range(H):
            As[h * C:(h + 1) * C, h] = a_s[h]
            Ad[h * C:(h + 1) * C, h] = a_d[h]
        return np.concatenate([W, W @ As, W @ Ad], axis=1)

    W2p = wprime(W2, as2, ad2, H2, C2)                # [512, 520]
    W3p = wprime(W3, as3, ad3, H3, C3)                # [512, 18]
    W2c = np.ascontiguousarray(W2p.reshape(4, 128, 520))
    W3c = np.ascontiguousarray(W3p.reshape(4, 128, 18))

    def bc(v, F):
        t = np.zeros((128, F), np.float32); t[:, :] = v[None, :F]; return t

    consts = dict(
        W2c=W2c.astype(BF), W3c=W3c.astype(BF),
        Z1A=Z1A.astype(BF), Z1B=Z1B.astype(BF),
        b1t=bc(b1, 512), g1t=bc(g1, 512), be1t=bc(be1, 512),
        b2t=bc(b2, 512), g2t=bc(g2, 512), be2t=bc(be2, 512),
        b3t=bc(b3, 16), g3t=bc(g3, 16), be3t=bc(be3, 16),
        iotaF=np.tile(np.arange(128, dtype=np.float32), (128, 1)).astype(BF),
        ident=np.eye(128, dtype=np.float32).astype(BF),
    )

    # ---- per-core edge bucketing (non-self edges only) ----
    es, ed = ei[0], ei[1]
    core_of = ed // NPC
    dr = ed % NPC
    grp_of = dr // 128
    dloc = dr % 128
    sc = es // NPC
    sr = es % NPC
    ti = np.searchsorted(np.array(TLO[1:]), sr, side='right')     # table id 0..2
    tlo = np.array(TLO)[ti]
    gidx = sc * np.array(TSPAN)[ti] + (sr - tlo)                  # row in table

    order = np.lexsort((gidx, ti, grp_of, core_of))
    core_s = core_of[order]; grp_s = grp_of[order]
    ti_s = ti[order]; gidx_s = gidx[order]
    dloc_s = dloc[order]

    key = (core_s * NG + grp_s) * NTAB + ti_s
    bounds = np.searchsorted(key, np.arange(NC * NG * NTAB + 1))
    cntT = (bounds[1:] - bounds[:-1]).reshape(NC, NG, NTAB)
    Tn = np.maximum(1, -(-cntT.max(axis=0) // 128))               # [NG, NTAB]
    Tt = Tn.sum(axis=1)                                           # [NG]
    oN = np.zeros((NTAB, NG + 1), np.int64)
    for t in range(NTAB):
        oN[t, 1:] = np.cumsum(Tn[:, t])
    oT = np.concatenate([[0], np.cumsum(Tt)]).astype(np.int64)
    NTn = [int(oN[t, -1]) for t in range(NTAB)]
    NTT = int(oT[-1])

    percore = []
    for c in range(NC):
        idxN = [np.zeros((128, NTn[t] * 8), np.int16) for t in range(NTAB)]
        idxD = np.zeros((128, NTT * 8), np.int16)
        dlS = np.full((NTT, 128), 200.0, np.float32)
        for g in range(NG):
            r0 = g * 128
            tb = int(oT[g])
            for t in range(NTAB):
                k = (c * NG + g) * NTAB + t
                s, e = bounds[k], bounds[k + 1]
                n = e - s
                cap = int(Tn[g, t]) * 128
                assert n <= cap
                flat = np.zeros(cap, np.int64)
                flat[:n] = gidx_s[s:e]
                dl = np.full(cap, 200.0, np.float32)
                dl[:n] = dloc_s[s:e]
                o = int(oN[t, g])
                idxN[t][:, o * 8:(o + int(Tn[g, t])) * 8] = _wrap_idx(flat)
                dlS[tb:tb + int(Tn[g, t])] = dl.reshape(int(Tn[g, t]), 128)
                dfl = np.full(cap, float(r0), np.float32)
                dfl[:n] = r0 + dloc_s[s:e]
                idxD[:, tb * 8:(tb + int(Tn[g, t])) * 8] = _wrap_idx(dfl.astype(np.int64))
                tb += int(Tn[g, t])
        lo, hi = c * NPC, (c + 1) * NPC
        cntc = np.zeros((NPCP, TILE), np.float32)
        cntc[:NPC] = cnt[lo:hi]
        cntc[NPC:, 0] = 1.0
        Ec = np.zeros((NPCP, H1 * TILE), np.float32)
        Ec[:NPC] = E_tab[x[lo:hi]].reshape(NPC, H1 * TILE)
        Ec[NPC:] = 1.0
        batchc = np.full((NPCP, 1), 200.0, np.float32)
        batchc[:NPC, 0] = batch[lo:hi]
        percore.append(dict(
            cntc=cntc.astype(BF), Ec=Ec.astype(BF),
            batchc=batchc,
            idx0=idxN[0], idx1=idxN[1], idx2=idxN[2], idxD=idxD,
            dlS=np.ascontiguousarray(dlS.T).astype(BF),   # [128, NTT]
        ))

    meta = dict(Tn=Tn.tolist(), Tt=Tt.tolist(),
                oN=oN.tolist(), oT=oT.tolist(),
                NTn=NTn, NTT=NTT)
    host = dict(fcW1=np.asarray(inputs['fcW1'], np.float32),
                fcb1=np.asarray(inputs['fcb1'], np.float32),
                fcW2=np.asarray(inputs['fcW2'], np.float32),
                fcb2=np.asarray(inputs['fcb2'], np.float32))
    return consts, percore, meta, host


def layer_norm_elu(nc, pool, y, g_t, be_t, F, epsc):
    """In SBUF: y f32 [128,F] -> elu(LN(y)*g+be) f32. Returns new tile."""
    s1 = pool.tile([128, 1], F32, tag="ln_s1")
    nc.vector.tensor_reduce(out=s1[:], in_=y[:], axis=AX.X, op=ALU.add)
    m2 = pool.tile([128, 1], F32, tag="ln_m2")
    nc.vector.tensor_scalar_mul(out=m2[:], in0=s1[:], scalar1=-1.0 / F)
    sq = pool.tile([128, F], F32, tag="ln_sq")
    ss = pool.tile([128, 1], F32, tag="ln_ss")
    nc.scalar.activation(out=sq[:], in_=y[:], func=ACT.Square, bias=m2[:, :1],
                         accum_out=ss[:])
    sd = pool.tile([128, 1], F32, tag="ln_sd")
    nc.scalar.activation(out=sd[:], in_=ss[:], func=ACT.Sqrt, bias=epsc[:, :1], scale=1.0 / F)
    rs = pool.tile([128, 1], F32, tag="ln_rs")
    nc.vector.reciprocal(out=rs[:], in_=sd[:])
    nc.vector.tensor_scalar(out=y[:], in0=y[:], scalar1=m2[:, :1], scalar2=rs[:, :1],
                            op0=ALU.add, op1=ALU.mult)
    nc.vector.tensor_tensor(out=y[:], in0=y[:], in1=g_t[:, :F], op=ALU.mult)
    nc.vector.tensor_tensor(out=y[:], in0=y[:], in1=be_t[:, :F], op=ALU.add)
    # ELU = max(x,0) + exp(min(x,0)) - 1
    nc.vector.tensor_scalar_min(out=sq[:], in0=y[:], scalar1=0.0)
    nc.scalar.activation(out=sq[:], in_=sq[:], func=ACT.Exp)
    h = pool.tile([128, F], F32, tag="elu_h")
    nc.vector.tensor_scalar(out=h[:], in0=y[:], scalar1=0.0, scalar2=-1.0,
                            op0=ALU.max, op1=ALU.add)
    nc.vector.tensor_tensor(out=h[:], in0=h[:], in1=sq[:], op=ALU.add)
    return h


def build(meta, phases=3):
    Tn, Tt = meta['Tn'], meta['Tt']
    oN, oT = meta['oN'], meta['oT']
    NTn, NTT = meta['NTn'], meta['NTT']
    NC, G = NC_FULL, G_FULL
    TILE, H1 = 32, 8
    H2x = 4

    def tab_of(g):
        for t in range(NTAB):
            if g < TBOUND[t + 1]:
                return t, (g - TBOUND[t]) * 128
        raise AssertionError

    nc = bacc.Bacc("TRN2", num_devices=NC, num_swdge_queues=4)
    t_cnt = nc.dram_tensor("cntc", [NPCP, TILE], BF16, kind="ExternalInput")
    t_E = nc.dram_tensor("Ec", [NPCP, H1 * TILE], BF16, kind="ExternalInput")
    t_bat = nc.dram_tensor("batchc", [NPCP, 1], F32, kind="ExternalInput")
    t_iN = [nc.dram_tensor(f"idx{t}", [128, NTn[t] * 8], I16, kind="ExternalInput")
            for t in range(NTAB)]
    t_iD = nc.dram_tensor("idxD", [128, NTT * 8], I16, kind="ExternalInput")
    t_dl = nc.dram_tensor("dlS", [128, NTT], BF16, kind="ExternalInput")
    t_W2c = nc.dram_tensor("W2c", [4, 128, 520], BF16, kind="ExternalInput")
    t_W3c = nc.dram_tensor("W3c", [4, 128, 18], BF16, kind="ExternalInput")
    t_Z1A = nc.dram_tensor("Z1A", [128, 256], BF16, kind="ExternalInput")
    t_Z1B = nc.dram_tensor("Z1B", [128, 256], BF16, kind="ExternalInput")
    cn = {}
    for nm, sh in [("b1t", 512), ("g1t", 512), ("be1t", 512), ("b2t", 512),
                   ("g2t", 512), ("be2t", 512), ("b3t", 16), ("g3t", 16), ("be3t", 16)]:
        cn[nm] = nc.dram_tensor(nm, [128, sh], F32, kind="ExternalInput")
    t_iota = nc.dram_tensor("iotaF", [128, 128], BF16, kind="ExternalInput")
    t_id = nc.dram_tensor("ident", [128, 128], BF16, kind="ExternalInput")
    t_out = nc.dram_tensor("part", [G, 17], F32, kind="ExternalOutput")

    with tile.TileContext(nc) as tc:
        with tc.tile_pool(name="const", bufs=1) as cp, \
             tc.tile_pool(name="sb", bufs=2) as sb, \
             tc.tile_pool(name="gbuf", bufs=2) as gb, \
             tc.tile_pool(name="dbuf", bufs=2) as db, \
             tc.tile_pool(name="ps", bufs=1, space="PSUM") as ps, \
             tc.tile_pool(name="pst", bufs=2, space="PSUM") as pst, \
             tc.tile_pool(name="pacc", bufs=1, space="PSUM") as pacc, \
             tc.tile_pool(name="dram", bufs=1, space="DRAM") as dp:

            # ---- const loads ----
            C = {}
            for nm, src, shp in [("iotaF", t_iota, [128, 128]), ("ident", t_id, [128, 128]),
                                 ("Z1A", t_Z1A, [128, 256]), ("Z1B", t_Z1B, [128, 256])]:
                C[nm] = cp.tile(shp, BF16, tag="c_" + nm, name="c_" + nm)
                nc.sync.dma_start(out=C[nm][:], in_=src[:])
            for nm in cn:
                F = 512 if nm[-2] != '3' else 16
                C[nm] = cp.tile([128, F], F32, tag="c_" + nm, name="c_" + nm)
                nc.sync.dma_start(out=C[nm][:], in_=cn[nm][:])
            W2s = cp.tile([128, 4 * 520], BF16, name="W2s")
            nc.sync.dma_start(out=W2s[:].rearrange("p (a b) -> p a b", a=4),
                              in_=t_W2c[:].rearrange("a p b -> p a b"))
            W3s = cp.tile([128, 4 * 18], BF16, name="W3s")
            nc.sync.dma_start(out=W3s[:].rearrange("p (a b) -> p a b", a=4),
                              in_=t_W3c[:].rearrange("a p b -> p a b"))
            epsc = cp.tile([128, 1], F32, name="epsc")
            nc.vector.memset(epsc[:], EPS)

            def gq(out_ap, in_ap, idxs_ap, ni, elem):
                nc.gpsimd.dma_gather(
                    out_ap=out_ap, in_ap=in_ap, idxs_ap=idxs_ap,
                    num_idxs=ni, num_idxs_reg=ni, elem_size=elem,
                    single_packet=False)

            def fire():
                pass

            def await_gathers():
                pass

            IN = []
            for t in range(NTAB):
                it = cp.tile([128, NTn[t] * 8], I16, name=f"c_idx{t}")
                nc.sync.dma_start(out=it[:], in_=t_iN[t][:])
                IN.append(it)
            ID = cp.tile([128, NTT * 8], I16, name="c_idxD")
            nc.sync.dma_start(out=ID[:], in_=t_iD[:])
            DL = cp.tile([128, NTT], BF16, name="c_dl")
            nc.sync.dma_start(out=DL[:], in_=t_dl[:])

            rec2_sh = [dp.tile([TSPAN[t], REC2], BF16, name=f"rec2_sh{t}")
                       for t in range(NTAB)]
            rec2_full = [dp.tile([NC * TSPAN[t], REC2], BF16, addr_space="Shared",
                                 name=f"rec2_full{t}") for t in range(NTAB)]
            att2 = dp.tile([NPCP, REC3], BF16, name="att2")
            rec3_sh = [dp.tile([TSPAN[t], REC3], BF16, name=f"rec3_sh{t}")
                       for t in range(NTAB)]
            rec3_full = [dp.tile([NC * TSPAN[t], REC3], BF16, addr_space="Shared",
                                 name=f"rec3_full{t}") for t in range(NTAB)]
            att3 = dp.tile([NPCP, REC3], BF16, name="att3")

            # ================= L1 + phaseA(L2) =================
            for g in range(NG):
                r0 = g * 128
                tb, rk = tab_of(g)
                cg = sb.tile([128, TILE], BF16, tag="cg")
                nc.sync.dma_start(out=cg[:], in_=t_cnt[r0:r0 + 128, :])
                Eg = sb.tile([128, H1, TILE], BF16, tag="Eg")
                nc.sync.dma_start(out=Eg[:, :, :],
                                  in_=t_E[r0:r0 + 128, :].rearrange("p (h t) -> p h t", h=H1))
                M = sb.tile([128, H1, TILE], BF16, tag="M")
                nc.vector.tensor_tensor(out=M[:, :, :], in0=Eg[:, :, :],
                                        in1=cg[:, None, :].to_broadcast([128, H1, TILE]),
                                        op=ALU.mult)
                s = sb.tile([128, H1], F32, tag="s")
                nc.vector.tensor_reduce(out=s[:], in_=M[:, :, :], axis=AX.X, op=ALU.add)
                rs = sb.tile([128, H1], F32, tag="rs")
                nc.vector.reciprocal(out=rs[:], in_=s[:])
                P = sb.tile([128, H1, TILE], BF16, tag="P")
                nc.vector.tensor_tensor(out=P[:, :, :], in0=M[:, :, :],
                                        in1=rs[:, :, None].to_broadcast([128, H1, TILE]),
                                        op=ALU.mult)
                pO = ps.tile([128, 512], F32, tag="pacc_main", space="PSUM")
                for half in range(2):
                    ptp = pst.tile([128, 128], BF16, tag="tp_ps", space="PSUM")
                    nc.tensor.transpose(
                        out=ptp[:],
                        in_=P[:, half * 4:(half + 1) * 4, :].rearrange("p h t -> p (h t)"),
                        identity=C["ident"][:])
                    PT = sb.tile([128, 128], BF16, tag="PT")
                    nc.vector.tensor_copy(out=PT[:], in_=ptp[:])
                    nc.tensor.matmul(out=pO[:, half * 256:(half + 1) * 256], lhsT=PT[:],
                                     rhs=C["Z1A" if half == 0 else "Z1B"][:],
                                     start=True, stop=True)
                y = sb.tile([128, 512], F32, tag="y1")
                nc.vector.tensor_tensor(out=y[:], in0=pO[:], in1=C["b1t"][:], op=ALU.add)
                h1 = layer_norm_elu(nc, sb, y, C["g1t"], C["be1t"], 512, epsc)
                h1b = sb.tile([128, 512], BF16, tag="h1b")
                nc.vector.tensor_copy(out=h1b[:], in_=h1[:])
                z2p = ps.tile([128, 512], F32, tag="pz", space="PSUM")
                z2pb = ps.tile([128, 8], F32, tag="pzb", space="PSUM")
                for q in range(4):
                    ptp = pst.tile([128, 128], BF16, tag="tp_ps", space="PSUM")
                    nc.tensor.transpose(out=ptp[:], in_=h1b[:, q * 128:(q + 1) * 128],
                                        identity=C["ident"][:])
                    hT = sb.tile([128, 128], BF16, tag="hT")
                    nc.vector.tensor_copy(out=hT[:], in_=ptp[:])
                    nc.tensor.matmul(out=z2p[:], lhsT=hT[:], rhs=W2s[:, q * 520:q * 520 + 512],
                                     start=(q == 0), stop=(q == 3))
                    nc.tensor.matmul(out=z2pb[:], lhsT=hT[:], rhs=W2s[:, q * 520 + 512:(q + 1) * 520],
                                     start=(q == 0), stop=(q == 3))
                zs = sb.tile([128, REC2], BF16, tag="zs")
                nc.vector.memset(zs[:, 520:], 0.0)
                nc.vector.tensor_copy(out=zs[:, :512], in_=z2p[:])
                nc.vector.tensor_copy(out=zs[:, 512:520], in_=z2pb[:])
                nc.sync.dma_start(out=rec2_sh[tb][rk:rk + 128, :], in_=zs[:])
                nc.sync.dma_start(out=att2[r0:r0 + 128, :], in_=zs[:, 512:640])
                if g + 1 in TBOUND:
                    nc.gpsimd.collective_compute(
                        "AllGather", ALU.bypass, replica_groups=[list(range(NC))],
                        ins=[rec2_sh[tb].opt()], outs=[rec2_full[tb].opt()])

            # ================= L2 + phaseA(L3) =================
            for g in range(NG if phases >= 2 else 0):
                r0 = g * 128
                tb, rk = tab_of(g)
                T = Tt[g]
                o0 = oT[g]
                dl = DL[:, o0:o0 + T]
                zg = sb.tile([128, 520], BF16, tag="zg")
                nc.sync.dma_start(out=zg[:], in_=rec2_sh[tb][rk:rk + 128, 0:520])
                Gt = gb.tile([128, T, REC2], BF16, tag="G")
                tacc = 0
                for t in range(NTAB):
                    for c0 in range(0, Tn[g][t], MAXT):
                        cw = min(MAXT, Tn[g][t] - c0)
                        gq(Gt[:, tacc + c0:tacc + c0 + cw, :], rec2_full[t][:],
                           IN[t][:, (oN[t][g] + c0) * 8:(oN[t][g] + c0 + cw) * 8],
                           cw * 128, REC2)
                    tacc += Tn[g][t]
                fire()
                S = gb.tile([128, T, 128], BF16, tag="S")
                nc.vector.tensor_tensor(
                    out=S[:, :, :],
                    in0=C["iotaF"][:, None, :].to_broadcast([128, T, 128]),
                    in1=dl[:, :, None].to_broadcast([128, T, 128]),
                    op=ALU.is_equal)
                pAR = ps.tile([128, T * H2x], F32, tag="pAR", space="PSUM")
                for t in range(T):
                    ptp = pst.tile([128, 128], BF16, tag="tp_ps", space="PSUM")
                    nc.tensor.transpose(out=ptp[:], in_=S[:, t, :], identity=C["ident"][:])
                    STt = sb.tile([128, 128], BF16, tag="STt")
                    nc.vector.tensor_copy(out=STt[:], in_=ptp[:])
                    nc.tensor.matmul(out=pAR[:, t * H2x:(t + 1) * H2x], lhsT=STt[:],
                                     rhs=zg[:, 516:520], start=True, stop=True)
                eL = sb.tile([128, T, H2x], BF16, tag="eL")
                nc.vector.tensor_tensor(
                    out=eL[:, :, :], in0=Gt[:, :, 512:516],
                    in1=pAR[:].rearrange("p (t h) -> p t h", h=H2x),
                    op=ALU.add)
                eA = sb.tile([128, T * H2x], BF16, tag="eA")
                nc.vector.scalar_tensor_tensor(
                    out=eA[:], in0=eL[:, :, :].rearrange("p t h -> p (t h)"),
                    scalar=0.2, in1=eL[:, :, :].rearrange("p t h -> p (t h)"),
                    op0=ALU.mult, op1=ALU.max)
                EX = sb.tile([128, T * H2x], BF16, tag="EX")
                nc.scalar.activation(out=EX[:], in_=eA[:], func=ACT.Exp)
                nc.vector.tensor_tensor(
                    out=Gt[:, :, :512].rearrange("p t (h c) -> p t h c", h=H2x),
                    in0=Gt[:, :, :512].rearrange("p t (h c) -> p t h c", h=H2x),
                    in1=EX[:].rearrange("p (t h) -> p t h", h=H2x)[:, :, :, None]
                        .to_broadcast([128, T, H2x, 128]),
                    op=ALU.mult)
                pMain = ps.tile([128, 512], F32, tag="pacc_main", space="PSUM")
                pS = ps.tile([128, H2x], F32, tag="pacc_s", space="PSUM")
                for t in range(T):
                    nc.tensor.matmul(out=pMain[:], lhsT=S[:, t, :], rhs=Gt[:, t, :512],
                                     start=(t == 0), stop=(t == T - 1))
                    nc.tensor.matmul(out=pS[:], lhsT=S[:, t, :], rhs=EX[:, t * H2x:(t + 1) * H2x],
                                     start=(t == 0), stop=(t == T - 1))
                # self-loop
                eSl = sb.tile([128, H2x], BF16, tag="eSl")
                nc.vector.tensor_tensor(out=eSl[:], in0=zg[:, 512:516], in1=zg[:, 516:520], op=ALU.add)
                eSa = sb.tile([128, H2x], BF16, tag="eSa")
                nc.vector.scalar_tensor_tensor(out=eSa[:], in0=eSl[:], scalar=0.2,
                                               in1=eSl[:], op0=ALU.mult, op1=ALU.max)
                exS = sb.tile([128, H2x], BF16, tag="exS")
                nc.scalar.activation(out=exS[:], in_=eSa[:], func=ACT.Exp)
                selfz = sb.tile([128, 512], BF16, tag="selfz")
                nc.vector.tensor_tensor(
                    out=selfz[:].rearrange("p (h c) -> p h c", h=H2x),
                    in0=zg[:, :512].rearrange("p (h c) -> p h c", h=H2x),
                    in1=exS[:, :, None].to_broadcast([128, H2x, 128]), op=ALU.mult)
                selfc = sb.tile([128, 512], F32, tag="selfc")
                nc.vector.tensor_tensor(out=selfc[:], in0=pMain[:], in1=selfz[:], op=ALU.add)
                sS = sb.tile([128, H2x], F32, tag="sS")
                nc.vector.tensor_tensor(out=sS[:], in0=pS[:], in1=exS[:], op=ALU.add)
                rS = sb.tile([128, H2x], F32, tag="rS")
                nc.vector.reciprocal(out=rS[:], in_=sS[:])
                nc.vector.tensor_tensor(
                    out=selfc[:].rearrange("p (h c) -> p h c", h=H2x),
                    in0=selfc[:].rearrange("p (h c) -> p h c", h=H2x),
                    in1=rS[:, :, None].to_broadcast([128, H2x, 128]), op=ALU.mult)
                nc.vector.tensor_tensor(out=selfc[:], in0=selfc[:], in1=C["b2t"][:], op=ALU.add)
                h2 = layer_norm_elu(nc, sb, selfc, C["g2t"], C["be2t"], 512, epsc)
                h2b = sb.tile([128, 512], BF16, tag="h2b")
                nc.vector.tensor_copy(out=h2b[:], in_=h2[:])
                z3p = ps.tile([128, 18], F32, tag="pz", space="PSUM")
                for q in range(4):
                    ptp = pst.tile([128, 128], BF16, tag="tp_ps", space="PSUM")
                    nc.tensor.transpose(out=ptp[:], in_=h2b[:, q * 128:(q + 1) * 128],
                                        identity=C["ident"][:])
                    hT = sb.tile([128, 128], BF16, tag="hT")
                    nc.vector.tensor_copy(out=hT[:], in_=ptp[:])
                    nc.tensor.matmul(out=z3p[:], lhsT=hT[:], rhs=W3s[:, q * 18:(q + 1) * 18],
                                     start=(q == 0), stop=(q == 3))
                z3s = sb.tile([128, REC3], BF16, tag="z3s")
                nc.vector.memset(z3s[:, 18:], 0.0)
                nc.vector.tensor_copy(out=z3s[:, :18], in_=z3p[:])
                nc.sync.dma_start(out=rec3_sh[tb][rk:rk + 128, :], in_=z3s[:])
                nc.sync.dma_start(out=att3[r0:r0 + 128, :], in_=z3s[:])
                if g + 1 in TBOUND:
                    nc.gpsimd.collective_compute(
                        "AllGather", ALU.bypass, replica_groups=[list(range(NC))],
                        ins=[rec3_sh[tb].opt()], outs=[rec3_full[tb].opt()])

            # ================= L3 + pooling =================
            pPool = pacc.tile([128, 17], F32, tag="pPool", space="PSUM")
            for g in range(NG if phases >= 3 else 0):
                r0 = g * 128
                tb, rk = tab_of(g)
                T = Tt[g]
                o0 = oT[g]
                dl = DL[:, o0:o0 + T]
                zg = sb.tile([128, 18], BF16, tag="zg")
                nc.sync.dma_start(out=zg[:], in_=rec3_sh[tb][rk:rk + 128, 0:18])
                bg = sb.tile([128, 1], F32, tag="bg")
                nc.sync.dma_start(out=bg[:], in_=t_bat[r0:r0 + 128, :])
                Gt = gb.tile([128, T, REC3], BF16, tag="G")
                tacc = 0
                for t in range(NTAB):
                    for c0 in range(0, Tn[g][t], MAXT):
                        cw = min(MAXT, Tn[g][t] - c0)
                        gq(Gt[:, tacc + c0:tacc + c0 + cw, :], rec3_full[t][:],
                           IN[t][:, (oN[t][g] + c0) * 8:(oN[t][g] + c0 + cw) * 8],
                           cw * 128, REC3)
                    tacc += Tn[g][t]
                fire()
                S = gb.tile([128, T, 128], BF16, tag="S")
                nc.vector.tensor_tensor(
                    out=S[:, :, :],
                    in0=C["iotaF"][:, None, :].to_broadcast([128, T, 128]),
                    in1=dl[:, :, None].to_broadcast([128, T, 128]),
                    op=ALU.is_equal)
                pAR = ps.tile([128, T], F32, tag="pAR", space="PSUM")
                for t in range(T):
                    ptp = pst.tile([128, 128], BF16, tag="tp_ps", space="PSUM")
                    nc.tensor.transpose(out=ptp[:], in_=S[:, t, :], identity=C["ident"][:])
                    STt = sb.tile([128, 128], BF16, tag="STt")
                    nc.vector.tensor_copy(out=STt[:], in_=ptp[:])
                    nc.tensor.matmul(out=pAR[:, t:t + 1], lhsT=STt[:],
                                     rhs=zg[:, 17:18], start=True, stop=True)
                eL = sb.tile([128, T], BF16, tag="eL")
                nc.vector.tensor_tensor(out=eL[:], in0=Gt[:, :, 16],
                                        in1=pAR[:],
                                        op=ALU.add)
                eA = sb.tile([128, T], BF16, tag="eA")
                nc.vector.scalar_tensor_tensor(out=eA[:], in0=eL[:], scalar=0.2,
                                               in1=eL[:], op0=ALU.mult, op1=ALU.max)
                EX = sb.tile([128, T], BF16, tag="EX")
                nc.scalar.activation(out=EX[:], in_=eA[:], func=ACT.Exp)
                nc.vector.tensor_tensor(
                    out=Gt[:, :, :16], in0=Gt[:, :, :16],
                    in1=EX[:, :, None].to_broadcast([128, T, 16]), op=ALU.mult)
                nc.vector.tensor_copy(out=Gt[:, :, 16], in_=EX[:])
                pM = ps.tile([128, 17], F32, tag="pacc_main", space="PSUM")
                for t in range(T):
                    nc.tensor.matmul(out=pM[:], lhsT=S[:, t, :], rhs=Gt[:, t, :17],
                                     start=(t == 0), stop=(t == T - 1))
                eSl = sb.tile([128, 1], BF16, tag="eSl")
                nc.vector.tensor_tensor(out=eSl[:], in0=zg[:, 16:17], in1=zg[:, 17:18], op=ALU.add)
                eSa = sb.tile([128, 1], BF16, tag="eSa")
                nc.vector.scalar_tensor_tensor(out=eSa[:], in0=eSl[:], scalar=0.2,
                                               in1=eSl[:], op0=ALU.mult, op1=ALU.max)
                exS = sb.tile([128, 1], BF16, tag="exS")
                nc.scalar.activation(out=exS[:], in_=eSa[:], func=ACT.Exp)
                selfz = sb.tile([128, 16], BF16, tag="selfz")
                nc.vector.tensor_tensor(out=selfz[:], in0=zg[:, :16],
                                        in1=exS[:, :1].to_broadcast([128, 16]), op=ALU.mult)
                selfc = sb.tile([128, 16], F32, tag="selfc")
                nc.vector.tensor_tensor(out=selfc[:], in0=pM[:, :16], in1=selfz[:], op=ALU.add)
                sS = sb.tile([128, 1], F32, tag="sS")
                nc.vector.tensor_tensor(out=sS[:], in0=pM[:, 16:17], in1=exS[:], op=ALU.add)
                rS = sb.tile([128, 1], F32, tag="rS")
                nc.vector.reciprocal(out=rS[:], in_=sS[:])
                nc.vector.tensor_scalar(out=selfc[:], in0=selfc[:], scalar1=rS[:, :1],
                                        scalar2=None, op0=ALU.mult)
                nc.vector.tensor_tensor(out=selfc[:], in0=selfc[:], in1=C["b3t"][:], op=ALU.add)
                h3 = layer_norm_elu(nc, sb, selfc, C["g3t"], C["be3t"], 16, epsc)
                OB = sb.tile([128, G], BF16, tag="OB")
                nc.vector.tensor_tensor(
                    out=OB[:], in0=C["iotaF"][:, :G],
                    in1=bg[:, :1].to_broadcast([128, G]), op=ALU.is_equal)
                h3w = sb.tile([128, 17], BF16, tag="h3w")
                nc.vector.tensor_copy(out=h3w[:, :16], in_=h3[:])
                nc.vector.memset(h3w[:, 16:17], 1.0)
                nc.tensor.matmul(out=pPool[:G, :17], lhsT=OB[:], rhs=h3w[:],
                                 start=(g == 0), stop=(g == NG - 1))
            po = sb.tile([128, 17], F32, tag="po")
            if phases >= 3:
                nc.vector.tensor_copy(out=po[:G, :], in_=pPool[:G, :])
            else:
                nc.vector.memset(po[:, :], 0.0)
            nc.sync.dma_start(out=t_out[:, :], in_=po[:G, :])
    nc.finalize()
    return nc


_CACHE = {}


def kernel(**inputs):
    consts, percore, meta, host = host_prep(inputs)
    key = tuple(tuple(r) for r in meta['Tn'])
    if key not in _CACHE:
        _CACHE[key] = build(meta)
    nc = _CACHE[key]
    in_maps = []
    for c in range(NC_FULL):
        m = dict(consts)
        m.update(percore[c])
        in_maps.append(m)
    from concourse.bass_utils import run_bass_kernel_spmd
    res = run_bass_kernel_spmd(nc, in_maps, core_ids=list(range(NC_FULL)))
    parts = np.stack([r["part"] for r in res.results])
    tot = parts.sum(axis=0)
    pooled = tot[:, :16] / np.maximum(tot[:, 16:17], 1.0)
    h = np.maximum(pooled @ host['fcW1'] + host['fcb1'], 0.0)
    return (h @ host['fcW2'] + host['fcb2']).astype(np.float32)
